# revision 1
# baseline (speedup 1.0000x reference)
"""Trainium2 Bass kernel for nn_Attention (Gaussian banded attention).

Math (reference):
    v = values @ input_weights.T                      # [B,L,D]
    probs[h,q,k] = N(k - q - off_h; std_h)            # Gaussian, depends on k-q only
    attended[b,h,q,:] = sum_k probs[h,q,k] v[b,k,h*pd:(h+1)*pd]
    out = attended_merged @ output_weight.T           # [B,L,D]

Key structural facts exploited:
  - probs is a banded Toeplitz matrix per head: nonzero only for
    k - q in [off - 6*std, off + 6*std] (6-sigma truncation, error ~1e-8).
    Widest band: std=8, off=-8 -> k-q in [-56, 40].
  - So attention is a narrow depthwise convolution along L; no [L,L] matmul.
  - Batch x L sharding is embarrassingly parallel given a halo of
    56 backward / 40 forward rows of the INPUT (v is a row-wise projection,
    zero rows project to zero since there is no bias).

Sharding: 8 cores = (B=2) x (4 chunks of 512 rows of L). Each core gets
x.T zero-padded to [1024, 640] (56 halo + 512 + 40 halo + 32 zero pad),
computes in [D, L]->[L, D]->[D, L] layouts on the TensorEngine in bf16,
and writes out.T [1024, 512] bf16 (host casts back to f32 on reassembly).
No collectives.

Cost-model performance (CoreSim, TRN2 timing): 38,609 ns single execution;
33,046 ns/iter steady state = TensorE 100% busy (gapless streaming floor).
"""

import math
from contextlib import ExitStack

import numpy as np
import ml_dtypes

import concourse.bass as bass
from concourse import mybir
from concourse.bass_utils import run_bass_kernel_spmd

# ---- NEFF disk cache (keyed by BIR hash) to avoid recompiling identical
# graphs in fresh processes ----
import hashlib
import os
import shutil

_NEFF_CACHE_DIR = os.environ.get("NEFF_CACHE_DIR", "/root/neff_cache")


def _install_neff_cache():
    import concourse.bass_utils as _bu
    import concourse.bass2jax as _b2j
    if getattr(_bu, "_neff_cache_installed", False):
        return
    orig = _bu.compile_bir_kernel

    def cached(bir_json, tmpdir, neff_name="file.neff"):
        cpath = None
        try:
            os.makedirs(_NEFF_CACHE_DIR, exist_ok=True)
            key = hashlib.sha256(bir_json).hexdigest()[:32]
            cpath = os.path.join(_NEFF_CACHE_DIR, f"{key}.neff")
            dst = os.path.join(tmpdir, neff_name)
            if os.path.exists(cpath):
                shutil.copy(cpath, dst)
                return dst
        except OSError:
            cpath = None  # cache unusable; plain compile below
        path = orig(bir_json, tmpdir, neff_name)
        if cpath is not None:
            try:
                shutil.copy(path, cpath)
            except OSError:
                pass
        return path

    _bu.compile_bir_kernel = cached
    _b2j.compile_bir_kernel = cached
    _bu._neff_cache_installed = True


_install_neff_cache()

# ---------------- problem constants (hardcoded per spec) ----------------
B, L, D = 2, 2048, 1024
H, PD = 8, 128
ATTN_STD = np.array([1.0, 2.0, 4.0, 8.0, 1.0, 2.0, 4.0, 8.0], dtype=np.float64)
ATTN_OFFSET = np.array([-1.0, -2.0, -4.0, -8.0, -1.0, -2.0, -4.0, -8.0], dtype=np.float64)

N_CORES = 8
CHUNK = 512            # output rows per core
HALO_L, HALO_R = 56, 40
LPAD = 640             # 56 + 512 + 40 = 608, padded to 5*128
LT = 5                 # l-tiles of v (640 / 128)
KT = 8                 # d tiles (1024 / 128)
NQ = CHUNK             # query columns per core

BF16 = mybir.dt.bfloat16
F32 = mybir.dt.float32

G1 = LT * 2            # proj1 groups: (l-tile, n-chunk) -> v
G2 = H                 # attention heads -> attendedT
G3 = KT                # proj2 d_out tiles -> outT
NPS = 4                # rotating PSUM banks


def gauss_toeplitz_table() -> np.ndarray:
    """tp[h, r, m] = g_h(r - (m - 512) - 56), shape [H, 128, 1024] bf16.

    For v-tile t (rows k' = 128t + r of padded-local v) the attention rhs is
    tp[h][:, 512-128t : 1024-128t] so that rhs[r, q'] = g_h(128t + r - q' - 56),
    which is probs[h, q, k].T in padded-local coordinates.
    """
    r = np.arange(128, dtype=np.float64)[:, None]
    m = np.arange(1024, dtype=np.float64)[None, :]
    delta = r - (m - 512.0) - 56.0  # = k - q
    tables = []
    for h in range(H):
        std, off = ATTN_STD[h], ATTN_OFFSET[h]
        z = (delta - off) / std
        g = np.exp(-0.5 * z * z) / (std * math.sqrt(2.0 * math.pi))
        g[np.abs(z) > 6.0] = 0.0
        tables.append(g)
    return np.stack(tables).astype(ml_dtypes.bfloat16)


def attn_windows(h: int):
    """Static (t, j0, j1) list: nonzero q-column window of v-tile t for head h,
    8-aligned. Coverage of [0,512) is guaranteed (window width > 128)."""
    std, off = int(ATTN_STD[h]), int(ATTN_OFFSET[h])
    wlo = -56 - off - 6 * std
    whi = 71 - off + 6 * std
    res = []
    for t in range(LT):
        j0 = max(0, 128 * t + wlo)
        j1 = min(NQ, 128 * t + whi + 1)
        if j0 >= j1:
            continue
        j0 = (j0 // 8) * 8
        j1 = min(NQ, ((j1 + 7) // 8) * 8)
        res.append((t, j0, j1))
    return res


def build_graph(iters: int = 1, banded: bool = True) -> bass.Bass:
    """One SPMD core program. iters>1 repeats the whole kernel (including
    DMAs) with monotonically increasing semaphore thresholds, for timing.

    Phase structure per iteration (PE program order):
      warmup: 3x N=256 + 1x N=184 discarded matmuls on a zeroed tile during
              the first DMA's latency window (p-state ramp off the critical
              path; the last MM is sized to land just past data-readiness);
      wave A: v[:, 0:512]  = x @ W1a  -- k-outer over psum banks 0-4 so the
              PE streams while the xt/w1a DMAs arrive;
      wave B: v[:, 512:1024] = x @ W1b -- k-inner, data resident, banks [5,6,7,0,1];
      ph2:    attendedT per head, banded Toeplitz windows, banks [2,3,4,5];
      ph3:    outT = W2 @ attendedT, banks [0,1,6,7] (so the last output
              copies gate nothing until wave B of the NEXT iteration).
    Copies: wave A -> vector, wave B -> scalar, ph2/ph3 alternate engines;
    xt/w1 double-buffered so iterations pipeline with zero PE gaps.
    """
    nc = bass.Bass()

    xt = nc.declare_dram_parameter("xt", [D, LPAD], BF16, isOutput=False)
    w1t = nc.declare_dram_parameter("w1t", [D, D], BF16, isOutput=False)
    w2t = nc.declare_dram_parameter("w2t", [D, D], BF16, isOutput=False)
    tp = nc.declare_dram_parameter("tp", [H, 128, 1024], BF16, isOutput=False)
    out = nc.declare_dram_parameter("out", [D, NQ], BF16, isOutput=True)

    xt_r = xt[:].rearrange("(o p) f -> p o f", p=128)    # [128, 8, 640]
    w1_r = w1t[:].rearrange("(o p) f -> p o f", p=128)   # [128, 8, 1024]
    w2_r = w2t[:].rearrange("(o p) f -> p o f", p=128)   # [128, 8, 1024]
    tp_r = tp[:].rearrange("h p f -> p h f")             # [128, 8, 1024]

    with ExitStack() as ctx:
        e = ctx.enter_context
        xt_sb = e(nc.sbuf_tensor("xt_sb", [128, 2, KT, LPAD], BF16))
        w1_sb = e(nc.sbuf_tensor("w1_sb", [128, 2, KT, D], BF16))
        w2_sb = e(nc.sbuf_tensor("w2_sb", [128, KT, D], BF16))
        TP0, TPW = (408, 240) if banded else (0, 1024)
        tp_sb = e(nc.sbuf_tensor("tp_sb", [128, H, TPW], BF16))
        tp_src = tp_r[:, :, TP0:TP0 + TPW]
        v_sb = e(nc.sbuf_tensor("v_sb", [128, LT, D], BF16))
        at_sb = e(nc.sbuf_tensor("at_sb", [128, H, NQ], BF16))
        o_sb = e(nc.sbuf_tensor("o_sb", [128, KT, NQ], BF16))
        zdum = e(nc.sbuf_tensor("zdum", [128, 384], BF16))
        ps = [e(nc.psum_tensor(f"ps{i}", [128, 512], F32)) for i in range(8)]

        sem_names = (["zd", "mmA", "mm1", "mm2", "mm3", "tp_d",
                      "cpA", "cpB", "cp2v", "cp2s", "cp3v", "cp3s"]
                     + [f"xt_d{k}b{p}" for k in range(KT) for p in (0, 1)]
                     + [f"{n}b{p}" for n in ("w1a_d0", "w1a_g1", "w1a_g2",
                                             "w1b_g1", "w1b_g2") for p in (0, 1)]
                     + ["w2_g1", "w2_g2"]
                     + [f"dmo{m}" for m in range(G3)])
        sems = {n: e(nc.semaphore(n)) for n in sem_names}

        WAVE_B_BANKS = [5, 6, 7, 0, 1]
        PH2_BANKS = [2, 3, 4, 5]
        PH3_BANKS = [0, 1, 6, 7]

        def cp2_sem(h):
            return sems["cp2v" if h % 2 == 0 else "cp2s"]

        def cp2_count(h, it):
            return it * 4 + h // 2 + 1

        def cp3_waits(m, it):
            """(sem, count) pairs proving ph3 group m is fully copied out."""
            s = sems["cp3v" if m % 2 == 0 else "cp3s"]
            return [(s, it * 4 + m // 2 + 1)]

        def cp3_sem(m):
            return cp3_waits(m, 0)[0][0]

        def cp3_count(m, it):
            return cp3_waits(m, it)[0][1]

        with nc.Block() as block:

            @block.sync
            def _(sync: bass.BassEngine):
                for it in range(iters):
                    buf = it % 2
                    if it > 1:
                        # xt/w1 buffer reuse: wave B (last reader) of iter it-2
                        sync.wait_ge(sems["mm1"], (it - 1) * LT)
                    def xt_dma(k):
                        sync.dma_start(out=xt_sb[:, buf, k, :],
                                       in_=xt_r[:, k, :]).then_inc(
                            sems[f"xt_d{k}b{buf}"], 16)

                    # schedule tuned so the HWDGE generator (625ns/DMA, shared)
                    # stays ahead of wave A's per-k consumption
                    xt_dma(0)
                    sync.dma_start(out=w1_sb[:, buf, 0, 0:512],
                                   in_=w1_r[:, 0, 0:512]).then_inc(
                        sems[f"w1a_d0b{buf}"], 16)
                    xt_dma(1)
                    sync.dma_start(out=w1_sb[:, buf, 1:4, 0:512],
                                   in_=w1_r[:, 1:4, 0:512]).then_inc(
                        sems[f"w1a_g1b{buf}"], 16)
                    xt_dma(2)
                    xt_dma(3)
                    sync.dma_start(out=w1_sb[:, buf, 4:8, 0:512],
                                   in_=w1_r[:, 4:8, 0:512]).then_inc(
                        sems[f"w1a_g2b{buf}"], 16)
                    for k in range(4, KT):
                        xt_dma(k)
                    sync.dma_start(out=w1_sb[:, buf, 0:4, 512:1024],
                                   in_=w1_r[:, 0:4, 512:1024]).then_inc(
                        sems[f"w1b_g1b{buf}"], 16)
                    sync.dma_start(out=w1_sb[:, buf, 4:8, 512:1024],
                                   in_=w1_r[:, 4:8, 512:1024]).then_inc(
                        sems[f"w1b_g2b{buf}"], 16)
                    if it == 0:
                        sync.dma_start(out=tp_sb[:], in_=tp_src).then_inc(
                            sems["tp_d"], 16)
                    if it > 0:
                        sync.wait_ge(sems["mm3"], it * G3)
                    sync.dma_start(out=w2_sb[:, 0:4, :],
                                   in_=w2_r[:, 0:4, :]).then_inc(sems["w2_g1"], 16)
                    sync.dma_start(out=w2_sb[:, 4:8, :],
                                   in_=w2_r[:, 4:8, :]).then_inc(sems["w2_g2"], 16)


            @block.tensor
            def _(tensor: bass.BassEngine):
                # HAM/p-state warmup: discarded matmuls into bank 0 while the
                # first input DMAs are in flight (wave A k=0 start=True clears)
                tensor.wait_ge(sems["zd"], 1)
                for _ in range(3):
                    tensor.matmul(ps[0][:, 0:256], zdum[:, 0:128],
                                  zdum[:, 128:384], start=True, stop=True)
                # final warmup trimmed to N=184 so the PE arrives at the wave-A
                # wait cluster just after data-readiness (the cost model defers
                # dispatch by ~1.6us if the PE arrives early - measured cliff)
                tensor.matmul(ps[0][:, 0:184], zdum[:, 0:128],
                              zdum[:, 128:312], start=True, stop=True)
                for it in range(iters):
                    buf = it % 2
                    # ---- wave A: v[:, 0:512], k-outer, banks 0-4 ----
                    # cross-iter bank WAR: last users in iter it-1 were
                    # ph2 (banks 2,3,4 via h=4,5,6; bank 5 via h=7) and
                    # ph3 (banks 6,7,0,1 via m=4,5,6,7)
                    nth = (it // 2 + 1) * 16  # per-parity DMA count
                    for k in range(KT):
                        tensor.wait_ge(sems[f"xt_d{k}b{buf}"], nth)
                        if k == 0:
                            tensor.wait_ge(sems[f"w1a_d0b{buf}"], nth)
                        elif k == 1:
                            tensor.wait_ge(sems[f"w1a_g1b{buf}"], nth)
                        elif k == 4:
                            tensor.wait_ge(sems[f"w1a_g2b{buf}"], nth)
                        for lt in range(LT):
                            if k == 0 and it > 0:
                                if lt == 0:
                                    for s, c in cp3_waits(4, it - 1):
                                        tensor.wait_ge(s, c)
                                elif lt == 1:
                                    for s, c in cp3_waits(5, it - 1):
                                        tensor.wait_ge(s, c)
                                else:  # banks 2,3,4 <- ph2 h=4,5,6
                                    tensor.wait_ge(cp2_sem(lt + 2),
                                                   cp2_count(lt + 2, it - 1))
                            mm = tensor.matmul(
                                ps[lt][:, :],
                                xt_sb[:, buf, k, 128 * lt:128 * lt + 128],
                                w1_sb[:, buf, k, 0:512],
                                start=(k == 0), stop=(k == KT - 1),
                            )
                            if k == KT - 1:
                                mm.then_inc(sems["mmA"])
                    # ---- wave B: v[:, 512:1024], k-inner, banks [5,6,7,0,1] ----
                    for lt in range(LT):
                        bank = ps[WAVE_B_BANKS[lt]]
                        if lt == 0:
                            if it > 0:  # bank 5 <- ph2 h=7 of prev iter
                                tensor.wait_ge(cp2_sem(7), cp2_count(7, it - 1))
                        elif lt == 1:
                            if it > 0:  # bank 6 <- ph3 m=6 of prev iter
                                for s, c in cp3_waits(6, it - 1):
                                    tensor.wait_ge(s, c)
                        elif lt == 2:
                            if it > 0:  # bank 7 <- ph3 m=7 of prev iter
                                for s, c in cp3_waits(7, it - 1):
                                    tensor.wait_ge(s, c)
                        elif lt == 3:
                            # bank 0 <- wave A lt=0 copy of this iter
                            tensor.wait_ge(sems["cpA"], it * LT + 1)
                        else:
                            # bank 1 <- wave A lt=1 copy of this iter
                            tensor.wait_ge(sems["cpA"], it * LT + 2)
                        for k in range(KT):
                            if lt == 0 and k == 0:
                                tensor.wait_ge(sems[f"w1b_g1b{buf}"], nth)
                            elif lt == 0 and k == 4:
                                tensor.wait_ge(sems[f"w1b_g2b{buf}"], nth)
                            mm = tensor.matmul(
                                bank[:, :],
                                xt_sb[:, buf, k, 128 * lt:128 * lt + 128],
                                w1_sb[:, buf, k, 512:1024],
                                start=(k == 0), stop=(k == KT - 1),
                            )
                            if k == KT - 1:
                                mm.then_inc(sems["mm1"])

                    # ---- phase 2: attendedT per head (banded), banks 0-3 ----
                    if it == 0:
                        tensor.wait_ge(sems["tp_d"], 16)
                    for h in range(G2):
                        bank = ps[PH2_BANKS[h % 4]]
                        # bank WAR: banks 2,3,4 <- wave A lt=2,3,4 copies;
                        # bank 5 <- wave B g=0 copy; h>=4 <- ph2 head h-4
                        if h == 0:
                            tensor.wait_ge(sems["cpA"], it * LT + 3)
                        elif h == 1:
                            tensor.wait_ge(sems["cpA"], it * LT + 4)
                        elif h == 2:
                            tensor.wait_ge(sems["cpA"], it * LT + 5)
                        elif h == 3:
                            tensor.wait_ge(sems["cpB"], it * LT + 1)
                        else:
                            tensor.wait_ge(cp2_sem(h - 4), cp2_count(h - 4, it))
                        windows = attn_windows(h) if banded else [
                            (t, 0, NQ) for t in range(LT)]
                        for wi, (t, j0, j1) in enumerate(windows):
                            # data: v tile t, n-chunk h//4
                            if h // 4 == 0:
                                tensor.wait_ge(sems["cpA"], it * LT + t + 1)
                            else:
                                tensor.wait_ge(sems["cpB"], it * LT + t + 1)
                            c0 = 512 - 128 * t + j0 - TP0
                            c1 = 512 - 128 * t + j1 - TP0
                            mm = tensor.matmul(
                                bank[:, j0:j1],
                                v_sb[:, t, 128 * h:128 * h + 128],
                                tp_sb[:, h, c0:c1],
                                start=(wi == 0), stop=(wi == len(windows) - 1),
                            )
                            if wi == len(windows) - 1:
                                mm.then_inc(sems["mm2"])

                    # ---- phase 3: outT = W2 @ attendedT, banks 0-3 ----
                    for m in range(G3):
                        bank = ps[PH3_BANKS[m % 4]]
                        # bank WAR: banks 6,7,0,1 <- wave B g=1,2,3,4 copies;
                        # m>=4 <- ph3 copy m-4
                        if m < 4:
                            # banks [0,1,6,7] freed by wave B groups [3,4,1,2]
                            tensor.wait_ge(sems["cpB"],
                                           it * LT + [4, 5, 2, 3][m])
                        else:
                            for s, c in cp3_waits(m - 4, it):
                                tensor.wait_ge(s, c)
                        for k in range(KT):
                            if m == 0:
                                tensor.wait_ge(cp2_sem(k), cp2_count(k, it))
                                if k == 0:
                                    tensor.wait_ge(sems["w2_g1"], (it + 1) * 16)
                                elif k == 4:
                                    tensor.wait_ge(sems["w2_g2"], (it + 1) * 16)
                            mm = tensor.matmul(
                                bank[:, :],
                                w2_sb[:, k, 128 * m:128 * m + 128],
                                at_sb[:, k, :],
                                start=(k == 0), stop=(k == KT - 1),
                            )
                            if k == KT - 1:
                                mm.then_inc(sems["mm3"])

            @block.vector
            def _(vector: bass.BassEngine):
                for it in range(iters):
                    # wave A copies: v[:, lt, 0:512], banks 0-4, after k=7 MM
                    for lt in range(LT):
                        vector.wait_ge(sems["mmA"], it * LT + lt + 1)
                        vector.tensor_copy(
                            out=v_sb[:, lt, 0:512], in_=ps[lt][:, :],
                        ).then_inc(sems["cpA"])
                    for h in range(G2):
                        if h % 2 != 0:
                            continue
                        vector.wait_ge(sems["mm2"], it * G2 + h + 1)
                        vector.tensor_copy(
                            out=at_sb[:, h, :], in_=ps[PH2_BANKS[h % 4]][:, :],
                        ).then_inc(sems["cp2v"])
                    for m in [0, 2, 4, 6]:
                        vector.wait_ge(sems["mm3"], it * G3 + m + 1)
                        if it > 0:
                            vector.wait_ge(sems[f"dmo{m}"], it * 16)
                        vector.tensor_copy(
                            out=o_sb[:, m, :],
                            in_=ps[PH3_BANKS[m % 4]][:, :],
                        ).then_inc(sems["cp3v"])

            @block.gpsimd
            def _(gpsimd: bass.BassEngine):
                # zero the PE-warmup tile before anything else
                gpsimd.memset(zdum[:], 0).then_inc(sems["zd"])
                for it in range(iters):
                    for m in range(G3):
                        for s, c in cp3_waits(m, it):
                            gpsimd.wait_ge(s, c)
                        gpsimd.dma_start(
                            out=out[128 * m:128 * m + 128, :],
                            in_=o_sb[:, m, :],
                        ).then_inc(sems[f"dmo{m}"], 16)
                for m in range(G3):
                    gpsimd.wait_ge(sems[f"dmo{m}"], iters * 16)

            @block.scalar
            def _(scalar: bass.BassEngine):
                for it in range(iters):
                    # wave B copies: v[:, lt, 512:1024] from banks [4,5,6,7,4]
                    for lt in range(LT):
                        scalar.wait_ge(sems["mm1"], it * LT + lt + 1)
                        scalar.copy(v_sb[:, lt, 512:1024],
                                    ps[WAVE_B_BANKS[lt]][:, :]).then_inc(sems["cpB"])
                    for h in range(G2):
                        if h % 2 != 1:
                            continue
                        scalar.wait_ge(sems["mm2"], it * G2 + h + 1)
                        scalar.copy(at_sb[:, h, :],
                                    ps[PH2_BANKS[h % 4]][:, :]).then_inc(sems["cp2s"])
                    for m in [1, 3, 5, 7]:
                        scalar.wait_ge(sems["mm3"], it * G3 + m + 1)
                        if it > 0:
                            scalar.wait_ge(sems[f"dmo{m}"], it * 16)
                        scalar.copy(o_sb[:, m, :],
                                    ps[PH3_BANKS[m % 4]][:, :]).then_inc(
                            sems["cp3s"])


    return nc


# ---------------- host side ----------------

_GRAPH_CACHE: dict = {}


def get_graph(iters: int = 1, banded: bool = True) -> bass.Bass:
    key = (iters, banded)
    if key not in _GRAPH_CACHE:
        _GRAPH_CACHE[key] = build_graph(iters, banded)
    return _GRAPH_CACHE[key]


class Runner:
    """Compile-once executor for one Bass graph across the 8 cores.

    Mirrors bass2jax.run_bass_via_pjrt but keeps the jitted callable so
    repeated invocations don't re-trace/re-compile.
    """

    def __init__(self, nc: bass.Bass, n_cores: int = N_CORES):
        import jax
        from jax.sharding import Mesh, PartitionSpec
        from jax.experimental.shard_map import shard_map
        from concourse import bass2jax, mybir as _mb

        bass2jax.install_neuronx_cc_hook()
        self.n_cores = n_cores

        partition_name = (nc.partition_id_tensor.name
                          if nc.partition_id_tensor else None)
        in_names, out_names, out_avals, zero_shapes = [], [], [], []
        for alloc in nc.m.functions[0].allocations:
            if not isinstance(alloc, _mb.MemoryLocationSet):
                continue
            name = alloc.memorylocations[0].name
            if alloc.kind == "ExternalInput":
                if name != partition_name:
                    in_names.append(name)
            elif alloc.kind == "ExternalOutput":
                out_names.append(name)
                shape = tuple(alloc.tensor_shape)
                dtype = _mb.dt.np(alloc.dtype)
                out_avals.append(jax.core.ShapedArray(shape, dtype))
                zero_shapes.append((shape, dtype))
        self.in_names = list(in_names)
        self.out_names = out_names
        self.out_avals = out_avals
        self.zero_shapes = zero_shapes
        n_params = len(in_names)
        all_names = in_names + out_names
        if partition_name is not None:
            all_names = all_names + [partition_name]

        def _body(*args):
            operands = list(args)
            if partition_name is not None:
                operands.append(bass2jax.partition_id_tensor())
            outs = bass2jax._bass_exec_p.bind(
                *operands,
                out_avals=tuple(out_avals),
                in_names=tuple(all_names),
                out_names=tuple(out_names),
                lowering_input_output_aliases=(),
                sim_require_finite=True,
                sim_require_nnan=True,
                nc=nc,
            )
            return tuple(outs)

        devices = jax.devices()[:n_cores]
        mesh = Mesh(np.asarray(devices), ("core",))
        self._mesh = mesh
        n_outs = len(out_names)
        self._fn = jax.jit(
            shard_map(_body, mesh=mesh,
                      in_specs=(PartitionSpec("core"),) * (n_params + n_outs),
                      out_specs=(PartitionSpec("core"),) * n_outs,
                      check_rep=False),
            donate_argnums=tuple(range(n_params, n_params + n_outs)),
            keep_unused=True,
        )

    def stage(self, in_maps):
        """device_put the concatenated inputs once; returns device arrays."""
        import jax
        concat_in = [
            np.concatenate([np.asarray(m[name]) for m in in_maps], axis=0)
            for name in self.in_names
        ]
        return [jax.device_put(a) for a in concat_in]

    def make_zeros(self):
        if not hasattr(self, "_zeros_fn"):
            import jax
            import jax.numpy as jnp
            from jax.sharding import NamedSharding, PartitionSpec
            shardings = tuple(
                NamedSharding(self._mesh, PartitionSpec("core"))
                for _ in self.zero_shapes)
            shapes = [((self.n_cores * s[0], *s[1:]), d)
                      for s, d in self.zero_shapes]

            def _mk():
                return tuple(jnp.zeros(sh, dt) for sh, dt in shapes)

            self._zeros_fn = jax.jit(_mk, out_shardings=shardings)
        return list(self._zeros_fn())

    def run_staged(self, dev_in, dev_zeros):
        return self._fn(*dev_in, *dev_zeros)

    def __call__(self, in_maps):
        out_arrs = self._fn(*self.stage(in_maps), *self.make_zeros())
        return [
            {name: np.asarray(out_arrs[i]).reshape(
                self.n_cores, *self.out_avals[i].shape)[c]
             for i, name in enumerate(self.out_names)}
            for c in range(self.n_cores)
        ]


_RUNNER_CACHE: dict = {}


def get_runner(iters: int = 1) -> "Runner":
    if iters not in _RUNNER_CACHE:
        _RUNNER_CACHE[iters] = Runner(get_graph(iters))
    return _RUNNER_CACHE[iters]


def make_in_maps(values: np.ndarray, input_weights: np.ndarray,
                 output_weight: np.ndarray) -> list:
    bf = ml_dtypes.bfloat16
    w1t = np.ascontiguousarray(input_weights.T).astype(bf)
    w2t = np.ascontiguousarray(output_weight.T).astype(bf)
    tpt = gauss_toeplitz_table()
    in_maps = []
    for core in range(N_CORES):
        b, c = divmod(core, 4)
        lo, hi = c * CHUNK - HALO_L, c * CHUNK + CHUNK + HALO_R
        src_lo, src_hi = max(lo, 0), min(hi, L)
        xt_pad = np.zeros((D, LPAD), dtype=bf)
        xt_pad[:, src_lo - lo:src_hi - lo] = values[b, src_lo:src_hi, :].T.astype(bf)
        in_maps.append({"xt": xt_pad, "w1t": w1t, "w2t": w2t, "tp": tpt})
    return in_maps


def assemble(results: list) -> np.ndarray:
    out = np.empty((B, L, D), dtype=np.float32)
    for core in range(N_CORES):
        b, c = divmod(core, 4)
        out[b, c * CHUNK:(c + 1) * CHUNK, :] = \
            results[core]["out"].T.astype(np.float32)
    return out


def kernel(values: np.ndarray, input_weights: np.ndarray,
           output_weight: np.ndarray) -> np.ndarray:
    in_maps = make_in_maps(values, input_weights, output_weight)
    try:
        return assemble(get_runner(1)(in_maps))
    except Exception:
        # fallback: canonical SPMD path (re-traces per call but always works)
        res = run_bass_kernel_spmd(get_graph(1), in_maps,
                                   core_ids=list(range(N_CORES)))
        return assemble(res.results)



# revision 2
# speedup vs baseline: 1.1066x; 1.1066x over previous
"""Trainium2 Bass kernel for nn_Attention (Gaussian banded attention).

Math (reference):
    v = values @ input_weights.T                      # [B,L,D]
    probs[h,q,k] = N(k - q - off_h; std_h)            # Gaussian, depends on k-q only
    attended[b,h,q,:] = sum_k probs[h,q,k] v[b,k,h*pd:(h+1)*pd]
    out = attended_merged @ output_weight.T           # [B,L,D]

Structural facts exploited:
  - probs is banded Toeplitz per head (6-sigma truncation) -> attention is a
    narrow depthwise conv along L, done as windowed matmuls vs a 128x1024
    Toeplitz table. Batch x L sharding is embarrassingly parallel with a
    56/40-row input halo (8 cores = 2 batches x 4 chunks of 512 rows).
  - The two dense 1024x1024 projections dominate PE time. They run as
    fp8e4(DoubleRow) matmuls: each instruction contracts 2x128 rows at
    0.5 cycles/row = 4x bf16 throughput. Full bf16-level precision is kept
    by splitting each operand x = hi + lo (both fp8) and accumulating
    three of the four cross terms in PSUM (hi*hi + lo*hi + hi*lo); the
    dropped lo*lo term is ~0.1% relative. Net projection cost: 6/8 of bf16.
  - Weights are pre-scaled by 256 (fp8e4 subnormal cutoff is 2^-6; raw
    weights have sigma 0.02), the Gaussian table by 4/256, and the host
    divides the output by 1024. All scales are powers of two (exact).

Phase structure per iteration (PE program order):
  warmup: discarded matmuls during the first DMA latency window;
  wave A: v[:, 0:512]   fp8 DoubleRow, banks 0-4 (bank=lt), kp-outer with
          passes HH,LH interleaved per kp and HL trailing (DMA streaming);
  wave B: v[:, 512:1024] banks [5,6,7,0,1], lt-outer, 12 matmuls/group;
  ph2:    attendedT per head, bf16 banded Toeplitz windows, banks [2,3,4,5];
          PSUM->SBUF copy to bf16 (DVE evens / Act odds) releases banks;
          fp8 hi/lo split runs in SBUF on DVE (evens) and Pool (odds);
  ph3:    outT = W2' @ attendedT, all 8 banks (bank=m); hi-consuming passes
          (HH then HL) kp-outer first, lo-consuming pass (LH) m-outer last
          so the fp8 splits hide behind ~7us of matmuls.
"""

import math
from contextlib import ExitStack

import numpy as np
import ml_dtypes

import concourse.bass as bass
from concourse import mybir
from concourse.bass_utils import run_bass_kernel_spmd

# ---- NEFF disk cache (keyed by BIR hash) to avoid recompiling identical
# graphs in fresh processes ----
import hashlib
import os
import shutil

_NEFF_CACHE_DIR = os.environ.get("NEFF_CACHE_DIR", "/root/neff_cache")


def _install_neff_cache():
    import concourse.bass_utils as _bu
    import concourse.bass2jax as _b2j
    if getattr(_bu, "_neff_cache_installed", False):
        return
    orig = _bu.compile_bir_kernel

    def cached(bir_json, tmpdir, neff_name="file.neff"):
        cpath = None
        try:
            os.makedirs(_NEFF_CACHE_DIR, exist_ok=True)
            key = hashlib.sha256(bir_json).hexdigest()[:32]
            cpath = os.path.join(_NEFF_CACHE_DIR, f"{key}.neff")
            dst = os.path.join(tmpdir, neff_name)
            if os.path.exists(cpath):
                shutil.copy(cpath, dst)
                return dst
        except OSError:
            cpath = None  # cache unusable; plain compile below
        path = orig(bir_json, tmpdir, neff_name)
        if cpath is not None:
            try:
                shutil.copy(path, cpath)
            except OSError:
                pass
        return path

    _bu.compile_bir_kernel = cached
    _b2j.compile_bir_kernel = cached
    _bu._neff_cache_installed = True


_install_neff_cache()

# ---------------- problem constants (hardcoded per spec) ----------------
B, L, D = 2, 2048, 1024
H, PD = 8, 128
ATTN_STD = np.array([1.0, 2.0, 4.0, 8.0, 1.0, 2.0, 4.0, 8.0], dtype=np.float64)
ATTN_OFFSET = np.array([-1.0, -2.0, -4.0, -8.0, -1.0, -2.0, -4.0, -8.0], dtype=np.float64)

N_CORES = 8
CHUNK = 512            # output rows per core
HALO_L, HALO_R = 56, 40
LPAD = 640             # 56 + 512 + 40 = 608, padded to 5*128
LT = 5                 # l-tiles of v (640 / 128)
KT = 8                 # d tiles (1024 / 128)
KP = 4                 # DoubleRow k-pairs (1024 / 256)
NQ = CHUNK             # query columns per core

BF16 = mybir.dt.bfloat16
F8 = mybir.dt.float8e4
F32 = mybir.dt.float32
DR = mybir.MatmulPerfMode.DoubleRow
NPF8 = ml_dtypes.float8_e4m3
NPBF = ml_dtypes.bfloat16

# power-of-two scales: weights *256 (clear fp8 subnormals), Gaussian table
# *4/256 (S1 cancel + attended into fp8 sweet spot), host output /(4*256)
S_W = 256.0
S_A = 4.0
OUT_DESCALE = 1.0 / (S_A * S_W)

WB = [5, 6, 7, 0, 1]       # wave B bank per lt
PH2_BANKS = [2, 3, 4, 5]   # ph2 bank = PH2_BANKS[h % 4]


def gauss_toeplitz_table() -> np.ndarray:
    """tp[h, r, m] = g_h(r - (m - 512) - 56) * S_A/S_W, shape [H,128,1024] bf16.

    For v-tile t (rows k' = 128t + r of padded-local v) the attention rhs is
    tp[h][:, 512-128t : 1024-128t] so that rhs[r, q'] = g_h(128t + r - q' - 56),
    which is probs[h, q, k].T in padded-local coordinates.
    """
    r = np.arange(128, dtype=np.float64)[:, None]
    m = np.arange(1024, dtype=np.float64)[None, :]
    delta = r - (m - 512.0) - 56.0  # = k - q
    tables = []
    for h in range(H):
        std, off = ATTN_STD[h], ATTN_OFFSET[h]
        z = (delta - off) / std
        g = np.exp(-0.5 * z * z) / (std * math.sqrt(2.0 * math.pi))
        g[np.abs(z) > 6.0] = 0.0
        tables.append(g * (S_A / S_W))
    return np.stack(tables).astype(NPBF)


def attn_windows(h: int):
    """Static (t, j0, j1) list: nonzero q-column window of v-tile t for head h,
    8-aligned. Coverage of [0,512) is guaranteed (window width > 128)."""
    std, off = int(ATTN_STD[h]), int(ATTN_OFFSET[h])
    wlo = -56 - off - 6 * std
    whi = 71 - off + 6 * std
    res = []
    for t in range(LT):
        j0 = max(0, 128 * t + wlo)
        j1 = min(NQ, 128 * t + whi + 1)
        if j0 >= j1:
            continue
        j0 = (j0 // 8) * 8
        j1 = min(NQ, ((j1 + 7) // 8) * 8)
        res.append((t, j0, j1))
    return res


def build_graph(iters: int = 1, banded: bool = True) -> bass.Bass:
    """One SPMD core program. iters>1 repeats the whole kernel (including
    DMAs) with monotonically increasing semaphore thresholds, for timing."""
    nc = bass.Bass()

    xt = nc.declare_dram_parameter("xt", [2, D, LPAD], F8, isOutput=False)
    w1 = nc.declare_dram_parameter("w1", [2, D, D], F8, isOutput=False)
    w2 = nc.declare_dram_parameter("w2", [2, D, D], F8, isOutput=False)
    tp = nc.declare_dram_parameter("tp", [H, 128, 1024], BF16, isOutput=False)
    out = nc.declare_dram_parameter("out", [D, NQ], BF16, isOutput=True)

    xt_r = xt[:].rearrange("s (o p) f -> p s o f", p=128)   # [128, 2, 8, 640]
    w1_r = w1[:].rearrange("s (o p) f -> p s o f", p=128)   # [128, 2, 8, 1024]
    w2_r = w2[:].rearrange("s (o p) f -> p s o f", p=128)   # [128, 2, 8, 1024]
    tp_r = tp[:].rearrange("h p f -> p h f")                # [128, 8, 1024]

    with ExitStack() as ctx:
        e = ctx.enter_context
        xt_sb = e(nc.sbuf_tensor("xt_sb", [128, 2, 2, KT, LPAD], F8))
        w1_sb = e(nc.sbuf_tensor("w1_sb", [128, 2, 2, KT, D], F8))
        w2_sb = e(nc.sbuf_tensor("w2_sb", [128, 2, KT, D], F8))
        TP0, TPW = (408, 240) if banded else (0, 1024)
        tp_sb = e(nc.sbuf_tensor("tp_sb", [128, H, TPW], BF16))
        tp_src = tp_r[:, :, TP0:TP0 + TPW]
        v_sb = e(nc.sbuf_tensor("v_sb", [128, LT, D], BF16))
        ab_sb = e(nc.sbuf_tensor("ab_sb", [128, H, NQ], BF16))   # attended bf16
        a8_sb = e(nc.sbuf_tensor("a8_sb", [128, 2, KT, NQ], F8))  # hi/lo fp8
        o_sb = e(nc.sbuf_tensor("o_sb", [128, KT, NQ], BF16))
        zdum = e(nc.sbuf_tensor("zdum", [128, 384], BF16))
        ps = [e(nc.psum_tensor(f"ps{i}", [128, 512], F32)) for i in range(8)]

        sem_names = (["zd", "mmA", "mm1", "mm2", "mm3", "tp_d",
                      "cpA", "cpB", "cp2v", "cp2s", "cp3v", "cp3s",
                      "phiV", "phiP", "ploV", "ploP",
                      "w2h0", "w2h1", "w2l0", "w2l1"]
                     + [f"xh{t}b{p}" for t in range(KP) for p in (0, 1)]
                     + [f"xl{t}b{p}" for t in range(KP) for p in (0, 1)]
                     + [f"wah{t}b{p}" for t in range(KP) for p in (0, 1)]
                     + [f"{n}{c}b{p}" for n in ("wal", "wbh", "wbl")
                        for c in (0, 1) for p in (0, 1)]
                     + [f"dmo{m}" for m in range(KT)])
        sems = {n: e(nc.semaphore(n)) for n in sem_names}

        def cp2_sem(h):
            return sems["cp2v" if h % 2 == 0 else "cp2s"]

        def cp2_count(h, it):
            return it * 4 + h // 2 + 1

        def cp3_sem(m):
            return sems["cp3v" if m % 2 == 0 else "cp3s"]

        def cp3_count(m, it):
            return it * 4 + m // 2 + 1

        with nc.Block() as block:

            @block.sync
            def _(sync: bass.BassEngine):
                for it in range(iters):
                    buf = it % 2
                    if it > 1:
                        # xt/w1 buffer reuse: wave B HL (last reader) of it-2
                        sync.wait_ge(sems["mm1"], (it - 1) * LT)

                    def dma(dst, src, sem):
                        sync.dma_start(out=dst, in_=src).then_inc(sems[sem], 16)

                    # interleaved to match wave A kp-outer consumption:
                    # per kp: HH needs xt_hi + w1a_hi, LH needs xt_lo
                    for t in range(KP):
                        dma(xt_sb[:, buf, 0, 2 * t:2 * t + 2, :],
                            xt_r[:, 0, 2 * t:2 * t + 2, :], f"xh{t}b{buf}")
                        dma(w1_sb[:, buf, 0, 2 * t:2 * t + 2, 0:512],
                            w1_r[:, 0, 2 * t:2 * t + 2, 0:512], f"wah{t}b{buf}")
                        dma(xt_sb[:, buf, 1, 2 * t:2 * t + 2, :],
                            xt_r[:, 1, 2 * t:2 * t + 2, :], f"xl{t}b{buf}")
                    for c in (0, 1):
                        dma(w1_sb[:, buf, 1, 4 * c:4 * c + 4, 0:512],
                            w1_r[:, 1, 4 * c:4 * c + 4, 0:512], f"wal{c}b{buf}")
                    for c in (0, 1):
                        dma(w1_sb[:, buf, 0, 4 * c:4 * c + 4, 512:1024],
                            w1_r[:, 0, 4 * c:4 * c + 4, 512:1024], f"wbh{c}b{buf}")
                    for c in (0, 1):
                        dma(w1_sb[:, buf, 1, 4 * c:4 * c + 4, 512:1024],
                            w1_r[:, 1, 4 * c:4 * c + 4, 512:1024], f"wbl{c}b{buf}")
                    if it == 0:
                        sync.dma_start(out=tp_sb[:], in_=tp_src).then_inc(
                            sems["tp_d"], 16)
                    if it > 0:
                        sync.wait_ge(sems["mm3"], it * KT)
                    dma(w2_sb[:, 0, 0:4, :], w2_r[:, 0, 0:4, :], "w2h0")
                    dma(w2_sb[:, 0, 4:8, :], w2_r[:, 0, 4:8, :], "w2h1")
                    dma(w2_sb[:, 1, 0:4, :], w2_r[:, 1, 0:4, :], "w2l0")
                    dma(w2_sb[:, 1, 4:8, :], w2_r[:, 1, 4:8, :], "w2l1")

            @block.tensor
            def _(tensor: bass.BassEngine):
                # HAM/p-state warmup: discarded matmuls into bank 0 while the
                # first input DMAs are in flight
                tensor.wait_ge(sems["zd"], 1)
                for _ in range(3):
                    tensor.matmul(ps[0][:, 0:256], zdum[:, 0:128],
                                  zdum[:, 128:384], start=True, stop=True)
                tensor.matmul(ps[0][:, 0:184], zdum[:, 0:128],
                              zdum[:, 128:312], start=True, stop=True)

                def dr_mm(bank, lhsT, rhs, start, stop):
                    return tensor.matmul(bank[:, :], lhsT, rhs,
                                         start=start, stop=stop, perf_mode=DR)

                for it in range(iters):
                    buf = it % 2
                    nth = (it // 2 + 1) * 16  # per-parity DMA count

                    # ---- wave A: v[:, 0:512], banks 0-4 (bank = lt) ----
                    # kp-outer; passes HH (x_hi*w_hi) and LH (x_lo*w_hi)
                    # interleaved per kp; HL (x_hi*w_lo) trails.
                    for t in range(KP):
                        tensor.wait_ge(sems[f"xh{t}b{buf}"], nth)
                        tensor.wait_ge(sems[f"wah{t}b{buf}"], nth)
                        for lt in range(LT):
                            if t == 0 and it > 0:
                                # bank lt <- ph3 m=lt copy of prev iter
                                tensor.wait_ge(cp3_sem(lt), cp3_count(lt, it - 1))
                            dr_mm(ps[lt],
                                  xt_sb[:, buf, 0, 2 * t:2 * t + 2,
                                        128 * lt:128 * lt + 128],
                                  w1_sb[:, buf, 0, 2 * t:2 * t + 2, 0:512],
                                  start=(t == 0), stop=False)
                        tensor.wait_ge(sems[f"xl{t}b{buf}"], nth)
                        for lt in range(LT):
                            dr_mm(ps[lt],
                                  xt_sb[:, buf, 1, 2 * t:2 * t + 2,
                                        128 * lt:128 * lt + 128],
                                  w1_sb[:, buf, 0, 2 * t:2 * t + 2, 0:512],
                                  start=False, stop=False)
                    for t in range(KP):
                        if t % 2 == 0:
                            tensor.wait_ge(sems[f"wal{t // 2}b{buf}"], nth)
                        for lt in range(LT):
                            mm = dr_mm(ps[lt],
                                       xt_sb[:, buf, 0, 2 * t:2 * t + 2,
                                             128 * lt:128 * lt + 128],
                                       w1_sb[:, buf, 1, 2 * t:2 * t + 2, 0:512],
                                       start=False, stop=(t == KP - 1))
                            if t == KP - 1:
                                mm.then_inc(sems["mmA"])

                    # ---- wave B: v[:, 512:1024], banks [5,6,7,0,1], lt-outer --
                    for lt in range(LT):
                        bank = ps[WB[lt]]
                        if it > 0 and lt == 0:
                            tensor.wait_ge(cp3_sem(5), cp3_count(5, it - 1))
                        elif it > 0 and lt == 1:
                            tensor.wait_ge(cp3_sem(6), cp3_count(6, it - 1))
                        elif it > 0 and lt == 2:
                            tensor.wait_ge(cp3_sem(7), cp3_count(7, it - 1))
                        elif lt == 3:
                            tensor.wait_ge(sems["cpA"], it * LT + 1)
                        elif lt == 4:
                            tensor.wait_ge(sems["cpA"], it * LT + 2)
                        for pas in range(3):  # HH, LH, HL
                            sx = 1 if pas == 1 else 0
                            sw = 1 if pas == 2 else 0
                            for t in range(KP):
                                if lt == 0 and pas == 0 and t % 2 == 0:
                                    tensor.wait_ge(sems[f"wbh{t // 2}b{buf}"], nth)
                                if lt == 0 and pas == 2 and t % 2 == 0:
                                    tensor.wait_ge(sems[f"wbl{t // 2}b{buf}"], nth)
                                mm = dr_mm(
                                    bank,
                                    xt_sb[:, buf, sx, 2 * t:2 * t + 2,
                                          128 * lt:128 * lt + 128],
                                    w1_sb[:, buf, sw, 2 * t:2 * t + 2, 512:1024],
                                    start=(pas == 0 and t == 0),
                                    stop=(pas == 2 and t == KP - 1))
                                if pas == 2 and t == KP - 1:
                                    mm.then_inc(sems["mm1"])

                    # ---- phase 2: attendedT per head (bf16 banded) ----
                    if it == 0:
                        tensor.wait_ge(sems["tp_d"], 16)
                    for h in range(H):
                        bank = ps[PH2_BANKS[h % 4]]
                        # bank WAR: banks 2,3,4 <- wave A lt=2,3,4 copies;
                        # bank 5 <- wave B lt=0 copy; h>=4 <- ph2 head h-4 copy
                        if h == 0:
                            tensor.wait_ge(sems["cpA"], it * LT + 3)
                        elif h == 1:
                            tensor.wait_ge(sems["cpA"], it * LT + 4)
                        elif h == 2:
                            tensor.wait_ge(sems["cpA"], it * LT + 5)
                        elif h == 3:
                            tensor.wait_ge(sems["cpB"], it * LT + 1)
                        else:
                            tensor.wait_ge(cp2_sem(h - 4), cp2_count(h - 4, it))
                        windows = attn_windows(h) if banded else [
                            (t, 0, NQ) for t in range(LT)]
                        for wi, (t, j0, j1) in enumerate(windows):
                            if h // 4 == 0:
                                tensor.wait_ge(sems["cpA"], it * LT + t + 1)
                            else:
                                tensor.wait_ge(sems["cpB"], it * LT + t + 1)
                            c0 = 512 - 128 * t + j0 - TP0
                            c1 = 512 - 128 * t + j1 - TP0
                            mm = tensor.matmul(
                                bank[:, j0:j1],
                                v_sb[:, t, 128 * h:128 * h + 128],
                                tp_sb[:, h, c0:c1],
                                start=(wi == 0), stop=(wi == len(windows) - 1),
                            )
                            if wi == len(windows) - 1:
                                mm.then_inc(sems["mm2"])

                    # ---- phase 3: outT = W2' @ attendedT, banks 0-7 ----
                    # stage 1: HH then HL, kp-outer (hi-consuming);
                    # stage 2: LH, m-outer (lo-consuming), closes groups.
                    for pas in range(2):  # 0=HH, 1=HL
                        sw = 1 if pas == 1 else 0
                        for t in range(KP):
                            if pas == 0:
                                tensor.wait_ge(sems["phiV"], it * 4 + t + 1)
                                tensor.wait_ge(sems["phiP"], it * 4 + t + 1)
                            if t % 2 == 0:
                                wsem = ("w2h0", "w2h1") if pas == 0 else \
                                       ("w2l0", "w2l1")
                                tensor.wait_ge(sems[wsem[t // 2]], (it + 1) * 16)
                            for m in range(KT):
                                if pas == 0 and t == 0:
                                    # bank WAR (first touch of bank m)
                                    if m == 0:
                                        tensor.wait_ge(sems["cpB"], it * LT + 4)
                                    elif m == 1:
                                        tensor.wait_ge(sems["cpB"], it * LT + 5)
                                    elif m in (2, 3, 4, 5):
                                        tensor.wait_ge(cp2_sem(m + 2),
                                                       cp2_count(m + 2, it))
                                    elif m == 6:
                                        tensor.wait_ge(sems["cpB"], it * LT + 2)
                                    else:
                                        tensor.wait_ge(sems["cpB"], it * LT + 3)
                                dr_mm(ps[m],
                                      w2_sb[:, sw, 2 * t:2 * t + 2,
                                            128 * m:128 * m + 128],
                                      a8_sb[:, 0, 2 * t:2 * t + 2, :],
                                      start=(pas == 0 and t == 0), stop=False)
                    for m in range(KT):
                        for t in range(KP):
                            if m == 0:
                                tensor.wait_ge(sems["ploV"], it * 4 + t + 1)
                                tensor.wait_ge(sems["ploP"], it * 4 + t + 1)
                            mm = dr_mm(ps[m],
                                       w2_sb[:, 0, 2 * t:2 * t + 2,
                                             128 * m:128 * m + 128],
                                       a8_sb[:, 1, 2 * t:2 * t + 2, :],
                                       start=False, stop=(t == KP - 1))
                            if t == KP - 1:
                                mm.then_inc(sems["mm3"])

            @block.vector
            def _(vector: bass.BassEngine):
                for it in range(iters):
                    # wave A copies: v[:, lt, 0:512] from banks 0-4
                    for lt in range(LT):
                        vector.wait_ge(sems["mmA"], it * LT + lt + 1)
                        vector.tensor_copy(
                            out=v_sb[:, lt, 0:512], in_=ps[lt][:, :],
                        ).then_inc(sems["cpA"])
                    # ph2 even heads: bf16 copy (frees bank) + fp8 hi split
                    for h in (0, 2, 4, 6):
                        vector.wait_ge(sems["mm2"], it * H + h + 1)
                        vector.tensor_copy(
                            out=ab_sb[:, h, :], in_=ps[PH2_BANKS[h % 4]][:, :],
                        ).then_inc(sems["cp2v"])
                        vector.tensor_copy(
                            out=a8_sb[:, 0, h, :], in_=ab_sb[:, h, :],
                        ).then_inc(sems["phiV"])
                    for h in (0, 2, 4, 6):
                        vector.tensor_sub(
                            a8_sb[:, 1, h, :], ab_sb[:, h, :], a8_sb[:, 0, h, :],
                        ).then_inc(sems["ploV"])
                    for m in (0, 2, 4, 6):
                        vector.wait_ge(sems["mm3"], it * KT + m + 1)
                        if it > 0:
                            vector.wait_ge(sems[f"dmo{m}"], it * 16)
                        vector.tensor_copy(
                            out=o_sb[:, m, :], in_=ps[m][:, :],
                        ).then_inc(sems["cp3v"])

            @block.gpsimd
            def _(gpsimd: bass.BassEngine):
                # zero the PE-warmup tile before anything else
                gpsimd.memset(zdum[:], 0).then_inc(sems["zd"])
                for it in range(iters):
                    # ph2 odd heads: fp8 hi then lo splits (SBUF only)
                    for h in (1, 3, 5, 7):
                        gpsimd.wait_ge(sems["cp2s"], it * 4 + h // 2 + 1)
                        gpsimd.tensor_copy(
                            out=a8_sb[:, 0, h, :], in_=ab_sb[:, h, :],
                        ).then_inc(sems["phiP"])
                    for h in (1, 3, 5, 7):
                        gpsimd.tensor_sub(
                            a8_sb[:, 1, h, :], ab_sb[:, h, :], a8_sb[:, 0, h, :],
                        ).then_inc(sems["ploP"])
                    for m in range(KT):
                        gpsimd.wait_ge(cp3_sem(m), cp3_count(m, it))
                        gpsimd.dma_start(
                            out=out[128 * m:128 * m + 128, :],
                            in_=o_sb[:, m, :],
                        ).then_inc(sems[f"dmo{m}"], 16)
                for m in range(KT):
                    gpsimd.wait_ge(sems[f"dmo{m}"], iters * 16)

            @block.scalar
            def _(scalar: bass.BassEngine):
                for it in range(iters):
                    # wave B copies: v[:, lt, 512:1024] from banks [5,6,7,0,1]
                    for lt in range(LT):
                        scalar.wait_ge(sems["mm1"], it * LT + lt + 1)
                        scalar.copy(v_sb[:, lt, 512:1024],
                                    ps[WB[lt]][:, :]).then_inc(sems["cpB"])
                    for h in (1, 3, 5, 7):
                        scalar.wait_ge(sems["mm2"], it * H + h + 1)
                        if it > 0:
                            # ab_sb[h] reuse: prev-iter lo split (Pool) done
                            scalar.wait_ge(sems["ploP"], (it - 1) * 4 + h // 2 + 1)
                        scalar.copy(ab_sb[:, h, :],
                                    ps[PH2_BANKS[h % 4]][:, :]).then_inc(sems["cp2s"])
                    for m in (1, 3, 5, 7):
                        scalar.wait_ge(sems["mm3"], it * KT + m + 1)
                        if it > 0:
                            scalar.wait_ge(sems[f"dmo{m}"], it * 16)
                        scalar.copy(o_sb[:, m, :],
                                    ps[m][:, :]).then_inc(sems["cp3s"])

    return nc


# ---------------- host side ----------------

_GRAPH_CACHE: dict = {}


def get_graph(iters: int = 1, banded: bool = True) -> bass.Bass:
    key = (iters, banded)
    if key not in _GRAPH_CACHE:
        _GRAPH_CACHE[key] = build_graph(iters, banded)
    return _GRAPH_CACHE[key]


class Runner:
    """Compile-once executor for one Bass graph across the 8 cores.

    Mirrors bass2jax.run_bass_via_pjrt but keeps the jitted callable so
    repeated invocations don't re-trace/re-compile.
    """

    def __init__(self, nc: bass.Bass, n_cores: int = N_CORES):
        import jax
        from jax.sharding import Mesh, PartitionSpec
        from jax.experimental.shard_map import shard_map
        from concourse import bass2jax, mybir as _mb

        bass2jax.install_neuronx_cc_hook()
        self.n_cores = n_cores

        partition_name = (nc.partition_id_tensor.name
                          if nc.partition_id_tensor else None)
        in_names, out_names, out_avals, zero_shapes = [], [], [], []
        for alloc in nc.m.functions[0].allocations:
            if not isinstance(alloc, _mb.MemoryLocationSet):
                continue
            name = alloc.memorylocations[0].name
            if alloc.kind == "ExternalInput":
                if name != partition_name:
                    in_names.append(name)
            elif alloc.kind == "ExternalOutput":
                out_names.append(name)
                shape = tuple(alloc.tensor_shape)
                dtype = _mb.dt.np(alloc.dtype)
                out_avals.append(jax.core.ShapedArray(shape, dtype))
                zero_shapes.append((shape, dtype))
        self.in_names = list(in_names)
        self.out_names = out_names
        self.out_avals = out_avals
        self.zero_shapes = zero_shapes
        n_params = len(in_names)
        all_names = in_names + out_names
        if partition_name is not None:
            all_names = all_names + [partition_name]

        def _body(*args):
            operands = list(args)
            if partition_name is not None:
                operands.append(bass2jax.partition_id_tensor())
            outs = bass2jax._bass_exec_p.bind(
                *operands,
                out_avals=tuple(out_avals),
                in_names=tuple(all_names),
                out_names=tuple(out_names),
                lowering_input_output_aliases=(),
                sim_require_finite=True,
                sim_require_nnan=True,
                nc=nc,
            )
            return tuple(outs)

        devices = jax.devices()[:n_cores]
        mesh = Mesh(np.asarray(devices), ("core",))
        self._mesh = mesh
        n_outs = len(out_names)
        self._fn = jax.jit(
            shard_map(_body, mesh=mesh,
                      in_specs=(PartitionSpec("core"),) * (n_params + n_outs),
                      out_specs=(PartitionSpec("core"),) * n_outs,
                      check_rep=False),
            donate_argnums=tuple(range(n_params, n_params + n_outs)),
            keep_unused=True,
        )

    def stage(self, in_maps):
        """device_put the concatenated inputs once; returns device arrays."""
        import jax
        concat_in = [
            np.concatenate([np.asarray(m[name]) for m in in_maps], axis=0)
            for name in self.in_names
        ]
        return [jax.device_put(a) for a in concat_in]

    def make_zeros(self):
        if not hasattr(self, "_zeros_fn"):
            import jax
            import jax.numpy as jnp
            from jax.sharding import NamedSharding, PartitionSpec
            shardings = tuple(
                NamedSharding(self._mesh, PartitionSpec("core"))
                for _ in self.zero_shapes)
            shapes = [((self.n_cores * s[0], *s[1:]), d)
                      for s, d in self.zero_shapes]

            def _mk():
                return tuple(jnp.zeros(sh, dt) for sh, dt in shapes)

            self._zeros_fn = jax.jit(_mk, out_shardings=shardings)
        return list(self._zeros_fn())

    def run_staged(self, dev_in, dev_zeros):
        return self._fn(*dev_in, *dev_zeros)

    def __call__(self, in_maps):
        out_arrs = self._fn(*self.stage(in_maps), *self.make_zeros())
        return [
            {name: np.asarray(out_arrs[i]).reshape(
                self.n_cores, *self.out_avals[i].shape)[c]
             for i, name in enumerate(self.out_names)}
            for c in range(self.n_cores)
        ]


_RUNNER_CACHE: dict = {}


def get_runner(iters: int = 1) -> "Runner":
    if iters not in _RUNNER_CACHE:
        _RUNNER_CACHE[iters] = Runner(get_graph(iters))
    return _RUNNER_CACHE[iters]


def _split8(a: np.ndarray) -> np.ndarray:
    """[2, ...] stack of (hi, lo) fp8e4 halves of a float32 array."""
    hi = a.astype(np.float32).astype(NPF8)
    lo = (a.astype(np.float32) - hi.astype(np.float32)).astype(NPF8)
    return np.stack([hi, lo])


def make_in_maps(values: np.ndarray, input_weights: np.ndarray,
                 output_weight: np.ndarray) -> list:
    w1s = _split8(np.ascontiguousarray(input_weights.T) * S_W)
    w2s = _split8(np.ascontiguousarray(output_weight.T) * S_W)
    tpt = gauss_toeplitz_table()
    in_maps = []
    for core in range(N_CORES):
        b, c = divmod(core, 4)
        lo, hi = c * CHUNK - HALO_L, c * CHUNK + CHUNK + HALO_R
        src_lo, src_hi = max(lo, 0), min(hi, L)
        xt_pad = np.zeros((D, LPAD), dtype=np.float32)
        xt_pad[:, src_lo - lo:src_hi - lo] = values[b, src_lo:src_hi, :].T
        in_maps.append({"xt": _split8(xt_pad), "w1": w1s, "w2": w2s, "tp": tpt})
    return in_maps


def assemble(results: list) -> np.ndarray:
    out = np.empty((B, L, D), dtype=np.float32)
    for core in range(N_CORES):
        b, c = divmod(core, 4)
        out[b, c * CHUNK:(c + 1) * CHUNK, :] = \
            results[core]["out"].T.astype(np.float32) * OUT_DESCALE
    return out


def kernel(values: np.ndarray, input_weights: np.ndarray,
           output_weight: np.ndarray) -> np.ndarray:
    in_maps = make_in_maps(values, input_weights, output_weight)
    try:
        return assemble(get_runner(1)(in_maps))
    except Exception:
        # fallback: canonical SPMD path (re-traces per call but always works)
        res = run_bass_kernel_spmd(get_graph(1), in_maps,
                                   core_ids=list(range(N_CORES)))
        return assemble(res.results)


# revision 31
# speedup vs baseline: 1.1636x; 1.0515x over previous
"""Trainium2 Bass kernel for nn_Attention (Gaussian banded attention).

Math (reference):
    v = values @ input_weights.T                      # [B,L,D]
    probs[h,q,k] = N(k - q - off_h; std_h)            # Gaussian, depends on k-q only
    attended[b,h,q,:] = sum_k probs[h,q,k] v[b,k,h*pd:(h+1)*pd]
    out = attended_merged @ output_weight.T           # [B,L,D]

Structural facts exploited:
  - probs is banded Toeplitz per head (6-sigma truncation) -> attention is a
    narrow depthwise conv along L, done as windowed matmuls vs a 128x1024
    Toeplitz table. Batch x L sharding is embarrassingly parallel with a
    56/40-row input halo (8 cores = 2 batches x 4 chunks of 512 rows).
  - The two dense 1024x1024 projections dominate PE time. They run as
    fp8e4(DoubleRow) matmuls: each instruction contracts 2x128 rows at
    0.5 cycles/row = 4x bf16 throughput. Full bf16-level precision is kept
    by splitting each operand x = hi + lo (both fp8) and accumulating
    three of the four cross terms in PSUM (hi*hi + lo*hi + hi*lo); the
    dropped lo*lo term is ~0.1% relative. Net projection cost: 6/8 of bf16.
  - Weights are pre-scaled by 256 (fp8e4 subnormal cutoff is 2^-6; raw
    weights have sigma 0.02), the Gaussian table by 4/256, and the host
    divides the output by 1024. All scales are powers of two (exact).

Phase structure per iteration (PE program order):
  warmup: discarded matmuls during the first DMA latency window;
  wave A: v[:, 0:512]   fp8 DoubleRow, banks 0-4 (bank=lt), kp-outer with
          passes HH,LH interleaved per kp and HL trailing (DMA streaming);
  wave B: v[:, 512:1024] banks [5,6,7,0,1], lt-outer, 12 matmuls/group;
  ph2:    attendedT per head, bf16 banded Toeplitz windows, banks [2,3,4,5];
          PSUM->SBUF copy to bf16 (DVE evens / Act odds) releases banks;
          fp8 hi/lo split runs in SBUF on DVE (evens) and Pool (odds);
  ph3:    outT = W2' @ attendedT, all 8 banks (bank=m); hi-consuming passes
          (HH then HL) kp-outer first, lo-consuming pass (LH) m-outer last
          so the fp8 splits hide behind ~7us of matmuls.
"""

import math
from contextlib import ExitStack

import numpy as np
import ml_dtypes

import concourse.bass as bass
from concourse import mybir
from concourse.bass_utils import run_bass_kernel_spmd

# ---- NEFF disk cache (keyed by BIR hash) to avoid recompiling identical
# graphs in fresh processes ----
import hashlib
import os
import shutil

_NEFF_CACHE_DIR = os.environ.get("NEFF_CACHE_DIR", "/root/neff_cache")


def _install_neff_cache():
    import concourse.bass_utils as _bu
    import concourse.bass2jax as _b2j
    if getattr(_bu, "_neff_cache_installed", False):
        return
    orig = _bu.compile_bir_kernel

    def cached(bir_json, tmpdir, neff_name="file.neff"):
        cpath = None
        try:
            os.makedirs(_NEFF_CACHE_DIR, exist_ok=True)
            key = hashlib.sha256(bir_json).hexdigest()[:32]
            cpath = os.path.join(_NEFF_CACHE_DIR, f"{key}.neff")
            dst = os.path.join(tmpdir, neff_name)
            if os.path.exists(cpath):
                shutil.copy(cpath, dst)
                return dst
        except OSError:
            cpath = None  # cache unusable; plain compile below
        path = orig(bir_json, tmpdir, neff_name)
        if cpath is not None:
            try:
                shutil.copy(path, cpath)
            except OSError:
                pass
        return path

    _bu.compile_bir_kernel = cached
    _b2j.compile_bir_kernel = cached
    _bu._neff_cache_installed = True


_install_neff_cache()

# ---------------- problem constants (hardcoded per spec) ----------------
B, L, D = 2, 2048, 1024
H, PD = 8, 128
ATTN_STD = np.array([1.0, 2.0, 4.0, 8.0, 1.0, 2.0, 4.0, 8.0], dtype=np.float64)
ATTN_OFFSET = np.array([-1.0, -2.0, -4.0, -8.0, -1.0, -2.0, -4.0, -8.0], dtype=np.float64)

N_CORES = 8
CHUNK = 512            # output rows per core
HALO_L, HALO_R = 56, 40
LPAD = 640             # 56 + 512 + 40 = 608, padded to 5*128
LT = 5                 # l-tiles of v (640 / 128)
KT = 8                 # d tiles (1024 / 128)
KP = 4                 # DoubleRow k-pairs (1024 / 256)
NQ = CHUNK             # query columns per core

BF16 = mybir.dt.bfloat16
F8 = mybir.dt.float8e4
F32 = mybir.dt.float32
DR = mybir.MatmulPerfMode.DoubleRow
NPF8 = ml_dtypes.float8_e4m3
NPBF = ml_dtypes.bfloat16

# power-of-two scales: weights *256 (clear fp8 subnormals), Gaussian table
# *4/256 (S1 cancel + attended into fp8 sweet spot), host output /(4*256)
S_W = 256.0
S_A = 4.0
OUT_DESCALE = 1.0 / (S_A * S_W)

TP0, TPW = 408, 256        # banded Toeplitz window (512B rows: no DMA penalty)

WB = [5, 6, 7, 0, 1]       # wave B bank per lt
PH2_BANKS = [2, 3, 4, 5]   # ph2 bank = PH2_BANKS[h % 4]

# p-state warmup: discarded matmul sizes (rows), burned during the first
# DMA latency window; tuned so the PE reaches the first wave-A wait just
# after data-readiness
WARMUP = (256, 256, 256, 184)


def gauss_toeplitz_table() -> np.ndarray:
    """tp[h, r, m] = g_h(r - (m - 512) - 56) * S_A/S_W, shape [H,128,1024] bf16.

    For v-tile t (rows k' = 128t + r of padded-local v) the attention rhs is
    tp[h][:, 512-128t : 1024-128t] so that rhs[r, q'] = g_h(128t + r - q' - 56),
    which is probs[h, q, k].T in padded-local coordinates.
    """
    r = np.arange(128, dtype=np.float64)[:, None]
    m = np.arange(1024, dtype=np.float64)[None, :]
    delta = r - (m - 512.0) - 56.0  # = k - q
    tables = []
    for h in range(H):
        std, off = ATTN_STD[h], ATTN_OFFSET[h]
        z = (delta - off) / std
        g = np.exp(-0.5 * z * z) / (std * math.sqrt(2.0 * math.pi))
        g[np.abs(z) > 6.0] = 0.0
        tables.append(g * (S_A / S_W))
    full = np.stack(tables).astype(NPBF)          # [H, 128, 1024]
    return np.ascontiguousarray(full[:, :, TP0:TP0 + TPW])


def attn_windows(h: int):
    """Static (t, j0, j1) list: nonzero q-column window of v-tile t for head h,
    8-aligned. Coverage of [0,512) is guaranteed (window width > 128)."""
    std, off = int(ATTN_STD[h]), int(ATTN_OFFSET[h])
    wlo = -56 - off - 6 * std
    whi = 71 - off + 6 * std
    res = []
    for t in range(LT):
        j0 = max(0, 128 * t + wlo)
        j1 = min(NQ, 128 * t + whi + 1)
        if j0 >= j1:
            continue
        j0 = (j0 // 8) * 8
        j1 = min(NQ, ((j1 + 7) // 8) * 8)
        res.append((t, j0, j1))
    return res


def build_graph(iters: int = 1, banded: bool = True) -> bass.Bass:
    """One SPMD core program. iters>1 repeats the whole kernel (including
    DMAs) with monotonically increasing semaphore thresholds, for timing."""
    nc = bass.Bass()

    xt = nc.declare_dram_parameter("xt", [2, D, LPAD], F8, isOutput=False)
    w1 = nc.declare_dram_parameter("w1", [2, D, D], F8, isOutput=False)
    w2 = nc.declare_dram_parameter("w2", [2, D, D], F8, isOutput=False)
    tp = nc.declare_dram_parameter("tp", [H, 128, TPW], BF16, isOutput=False)
    out = nc.declare_dram_parameter("out", [D, NQ], BF16, isOutput=True)

    xt_r = xt[:].rearrange("s (o p) f -> p s o f", p=128)   # [128, 2, 8, 640]
    w1_r = w1[:].rearrange("s (o p) f -> p s o f", p=128)   # [128, 2, 8, 1024]
    w2_r = w2[:].rearrange("s (o p) f -> p s o f", p=128)   # [128, 2, 8, 1024]
    tp_r = tp[:].rearrange("h p f -> p h f")                # [128, 8, 256]

    with ExitStack() as ctx:
        e = ctx.enter_context
        xt_sb = e(nc.sbuf_tensor("xt_sb", [128, 2, 2, KT, LPAD], F8))
        w1_sb = e(nc.sbuf_tensor("w1_sb", [128, 2, 2, KT, D], F8))
        w2_sb = e(nc.sbuf_tensor("w2_sb", [128, 2, KT, D], F8))
        tp_sb = e(nc.sbuf_tensor("tp_sb", [128, H, TPW], BF16))
        tp_src = tp_r[:, :, :]
        v_sb = e(nc.sbuf_tensor("v_sb", [128, LT, D], BF16))
        ab_sb = e(nc.sbuf_tensor("ab_sb", [128, H, NQ], BF16))   # attended bf16
        a8_sb = e(nc.sbuf_tensor("a8_sb", [128, 2, KT, NQ], F8))  # hi/lo fp8
        o_sb = e(nc.sbuf_tensor("o_sb", [128, KT, NQ], BF16))
        zdum = e(nc.sbuf_tensor("zdum", [128, 384], BF16))
        ps = [e(nc.psum_tensor(f"ps{i}", [128, 512], F32)) for i in range(8)]

        sem_names = (["zd", "mmA", "mm1", "mm2", "mm3", "tp_d",
                      "cpA", "cpB", "cp2v", "cp2s", "cp3v", "cp3s",
                      "phiV", "phiP", "ploV", "ploP", "w2h", "w2l"]
                     + [f"{n}b{p}" for n in ("xh0", "xh123", "xl01", "xl23",
                                             "wah0", "wah123", "wal01", "wal23",
                                             "wbh", "wbl") for p in (0, 1)]
                     + [f"dmo{m}" for m in range(KT)])
        sems = {n: e(nc.semaphore(n)) for n in sem_names}

        def cp2_sem(h):
            return sems["cp2v" if h % 2 == 0 else "cp2s"]

        def cp2_count(h, it):
            return it * 4 + h // 2 + 1

        def cp3_sem(m):
            return sems["cp3v" if m % 2 == 0 else "cp3s"]

        def cp3_count(m, it):
            return it * 4 + m // 2 + 1

        with nc.Block() as block:

            @block.sync
            def _(sync: bass.BassEngine):
                for it in range(iters):
                    buf = it % 2
                    if it > 1:
                        # xt/w1 buffer reuse: wave B HL (last reader) of it-2
                        sync.wait_ge(sems["mm1"], (it - 1) * LT)

                    def dma(dst, src, sem):
                        sync.dma_start(out=dst, in_=src).then_inc(sems[sem], 16)

                    # coarse chunks, issued in wave A consumption order
                    # (the shared HWDGE generator costs 625ns per issue)
                    dma(xt_sb[:, buf, 0, 0:2, :], xt_r[:, 0, 0:2, :],
                        f"xh0b{buf}")
                    dma(w1_sb[:, buf, 0, 0:2, 0:512], w1_r[:, 0, 0:2, 0:512],
                        f"wah0b{buf}")
                    dma(xt_sb[:, buf, 0, 2:8, :], xt_r[:, 0, 2:8, :],
                        f"xh123b{buf}")
                    dma(w1_sb[:, buf, 0, 2:8, 0:512], w1_r[:, 0, 2:8, 0:512],
                        f"wah123b{buf}")
                    dma(xt_sb[:, buf, 1, 0:4, :], xt_r[:, 1, 0:4, :],
                        f"xl01b{buf}")
                    dma(xt_sb[:, buf, 1, 4:8, :], xt_r[:, 1, 4:8, :],
                        f"xl23b{buf}")
                    dma(w1_sb[:, buf, 1, 0:4, 0:512], w1_r[:, 1, 0:4, 0:512],
                        f"wal01b{buf}")
                    dma(w1_sb[:, buf, 1, 4:8, 0:512], w1_r[:, 1, 4:8, 0:512],
                        f"wal23b{buf}")
                    dma(w1_sb[:, buf, 0, :, 512:1024], w1_r[:, 0, :, 512:1024],
                        f"wbhb{buf}")
                    dma(w1_sb[:, buf, 1, :, 512:1024], w1_r[:, 1, :, 512:1024],
                        f"wblb{buf}")
                    if it == 0:
                        sync.dma_start(out=tp_sb[:], in_=tp_src).then_inc(
                            sems["tp_d"], 16)
                    if it > 0:
                        sync.wait_ge(sems["mm3"], it * KT)
                    dma(w2_sb[:, 0, :, :], w2_r[:, 0, :, :], "w2h")
                    dma(w2_sb[:, 1, :, :], w2_r[:, 1, :, :], "w2l")

            @block.tensor
            def _(tensor: bass.BassEngine):
                # HAM/p-state warmup: discarded matmuls into bank 0 while the
                # first input DMAs are in flight
                tensor.wait_ge(sems["zd"], 1)
                for wn in WARMUP:
                    tensor.matmul(ps[0][:, 0:wn], zdum[:, 0:128],
                                  zdum[:, 128:128 + wn], start=True, stop=True)

                def dr_mm(bank, lhsT, rhs, start, stop):
                    return tensor.matmul(bank[:, :], lhsT, rhs,
                                         start=start, stop=stop, perf_mode=DR)

                for it in range(iters):
                    buf = it % 2
                    nth = (it // 2 + 1) * 16  # per-parity DMA count

                    # ---- wave A: v[:, 0:512], banks 0-4 (bank = lt) ----
                    # kp-outer; passes HH (x_hi*w_hi) and LH (x_lo*w_hi)
                    # interleaved per kp to match DMA arrival order; HL
                    # (x_hi*w_lo) trails.
                    for t in range(KP):
                        if t == 0:
                            tensor.wait_ge(sems[f"xh0b{buf}"], nth)
                            tensor.wait_ge(sems[f"wah0b{buf}"], nth)
                        elif t == 1:
                            tensor.wait_ge(sems[f"xh123b{buf}"], nth)
                            tensor.wait_ge(sems[f"wah123b{buf}"], nth)
                        for lt in range(LT):
                            if t == 0 and it > 0:
                                # bank lt <- ph3 m=lt copy of prev iter
                                tensor.wait_ge(cp3_sem(lt), cp3_count(lt, it - 1))
                            dr_mm(ps[lt],
                                  xt_sb[:, buf, 0, 2 * t:2 * t + 2,
                                        128 * lt:128 * lt + 128],
                                  w1_sb[:, buf, 0, 2 * t:2 * t + 2, 0:512],
                                  start=(t == 0), stop=False)
                    for t in range(KP):
                        if t == 0:
                            tensor.wait_ge(sems[f"xl01b{buf}"], nth)
                        elif t == 2:
                            tensor.wait_ge(sems[f"xl23b{buf}"], nth)
                        for lt in range(LT):
                            dr_mm(ps[lt],
                                  xt_sb[:, buf, 1, 2 * t:2 * t + 2,
                                        128 * lt:128 * lt + 128],
                                  w1_sb[:, buf, 0, 2 * t:2 * t + 2, 0:512],
                                  start=False, stop=False)
                    for t in range(KP):
                        if t == 0:
                            tensor.wait_ge(sems[f"wal01b{buf}"], nth)
                        elif t == 2:
                            tensor.wait_ge(sems[f"wal23b{buf}"], nth)
                        for lt in range(LT):
                            mm = dr_mm(ps[lt],
                                       xt_sb[:, buf, 0, 2 * t:2 * t + 2,
                                             128 * lt:128 * lt + 128],
                                       w1_sb[:, buf, 1, 2 * t:2 * t + 2, 0:512],
                                       start=False, stop=(t == KP - 1))
                            if t == KP - 1:
                                mm.then_inc(sems["mmA"])

                    # ---- wave B: v[:, 512:1024], banks [5,6,7,0,1], lt-outer --
                    for lt in range(LT):
                        bank = ps[WB[lt]]
                        if it > 0 and lt == 0:
                            tensor.wait_ge(cp3_sem(5), cp3_count(5, it - 1))
                        elif it > 0 and lt == 1:
                            tensor.wait_ge(cp3_sem(6), cp3_count(6, it - 1))
                        elif it > 0 and lt == 2:
                            tensor.wait_ge(cp3_sem(7), cp3_count(7, it - 1))
                        elif lt == 3:
                            tensor.wait_ge(sems["cpA"], it * LT + 1)
                        elif lt == 4:
                            tensor.wait_ge(sems["cpA"], it * LT + 2)
                        for pas in range(3):  # HH, LH, HL
                            sx = 1 if pas == 1 else 0
                            sw = 1 if pas == 2 else 0
                            for t in range(KP):
                                if lt == 0 and pas == 0 and t == 0:
                                    tensor.wait_ge(sems[f"wbhb{buf}"], nth)
                                if lt == 0 and pas == 2 and t == 0:
                                    tensor.wait_ge(sems[f"wblb{buf}"], nth)
                                mm = dr_mm(
                                    bank,
                                    xt_sb[:, buf, sx, 2 * t:2 * t + 2,
                                          128 * lt:128 * lt + 128],
                                    w1_sb[:, buf, sw, 2 * t:2 * t + 2, 512:1024],
                                    start=(pas == 0 and t == 0),
                                    stop=(pas == 2 and t == KP - 1))
                                if pas == 2 and t == KP - 1:
                                    mm.then_inc(sems["mm1"])

                    # ---- phase 2: attendedT per head (bf16 banded) ----
                    if it == 0:
                        tensor.wait_ge(sems["tp_d"], 16)
                    for h in range(H):
                        bank = ps[PH2_BANKS[h % 4]]
                        # bank WAR: banks 2,3,4 <- wave A lt=2,3,4 copies;
                        # bank 5 <- wave B lt=0 copy; h>=4 <- ph2 head h-4 copy
                        if h == 0:
                            tensor.wait_ge(sems["cpA"], it * LT + 3)
                        elif h == 1:
                            tensor.wait_ge(sems["cpA"], it * LT + 4)
                        elif h == 2:
                            tensor.wait_ge(sems["cpA"], it * LT + 5)
                        elif h == 3:
                            tensor.wait_ge(sems["cpB"], it * LT + 1)
                        else:
                            tensor.wait_ge(cp2_sem(h - 4), cp2_count(h - 4, it))
                        windows = attn_windows(h)
                        for wi, (t, j0, j1) in enumerate(windows):
                            if h // 4 == 0:
                                tensor.wait_ge(sems["cpA"], it * LT + t + 1)
                            else:
                                tensor.wait_ge(sems["cpB"], it * LT + t + 1)
                            c0 = 512 - 128 * t + j0 - TP0
                            c1 = 512 - 128 * t + j1 - TP0
                            mm = tensor.matmul(
                                bank[:, j0:j1],
                                v_sb[:, t, 128 * h:128 * h + 128],
                                tp_sb[:, h, c0:c1],
                                start=(wi == 0), stop=(wi == len(windows) - 1),
                            )
                            if wi == len(windows) - 1:
                                mm.then_inc(sems["mm2"])

                    # ---- phase 3: outT = W2' @ attendedT, banks 0-7 ----
                    # stage 1 (hi-consuming) is fully kp-outer so the at8-hi
                    # for pair t is needed only ~1.7us*t after ph3 starts,
                    # matching the pace the splits come off the copy engines.
                    # Per kp: HH/HL over m-set A (banks from wave B copies),
                    # then over m-set B (banks from ph2 copies, later).
                    # stage 2 (LH, lo-consuming) is m-outer and closes groups.
                    MSET_A = (0, 1, 6, 7)
                    MSET_B = (2, 3, 4, 5)
                    for t in range(KP):
                        tensor.wait_ge(sems["phiV"], it * 4 + t + 1)
                        tensor.wait_ge(sems["phiP"], it * 4 + t + 1)
                        if t == 0:
                            tensor.wait_ge(sems["w2h"], (it + 1) * 16)
                        for si, mset in enumerate((MSET_A, MSET_B)):
                            for pas in range(2):  # HH, HL
                                if t == 0 and si == 0 and pas == 1:
                                    tensor.wait_ge(sems["w2l"], (it + 1) * 16)
                                for m in mset:
                                    if t == 0 and pas == 0:
                                        # bank WAR (first touch of bank m)
                                        if m == 0:
                                            tensor.wait_ge(sems["cpB"],
                                                           it * LT + 4)
                                        elif m == 1:
                                            tensor.wait_ge(sems["cpB"],
                                                           it * LT + 5)
                                        elif m in MSET_B:
                                            tensor.wait_ge(cp2_sem(m + 2),
                                                           cp2_count(m + 2, it))
                                        elif m == 6:
                                            tensor.wait_ge(sems["cpB"],
                                                           it * LT + 2)
                                        else:
                                            tensor.wait_ge(sems["cpB"],
                                                           it * LT + 3)
                                    dr_mm(ps[m],
                                          w2_sb[:, pas, 2 * t:2 * t + 2,
                                                128 * m:128 * m + 128],
                                          a8_sb[:, 0, 2 * t:2 * t + 2, :],
                                          start=(t == 0 and pas == 0),
                                          stop=False)
                    for m in range(KT):  # LH pass, closes groups
                        for t in range(KP):
                            if m == 0:
                                tensor.wait_ge(sems["ploV"], it * 4 + t + 1)
                                tensor.wait_ge(sems["ploP"], it * 4 + t + 1)
                            mm = dr_mm(ps[m],
                                       w2_sb[:, 0, 2 * t:2 * t + 2,
                                             128 * m:128 * m + 128],
                                       a8_sb[:, 1, 2 * t:2 * t + 2, :],
                                       start=False, stop=(t == KP - 1))
                            if t == KP - 1:
                                mm.then_inc(sems["mm3"])

            @block.vector
            def _(vector: bass.BassEngine):
                for it in range(iters):
                    # wave A copies: v[:, lt, 0:512] from banks 0-4
                    for lt in range(LT):
                        vector.wait_ge(sems["mmA"], it * LT + lt + 1)
                        vector.tensor_copy(
                            out=v_sb[:, lt, 0:512], in_=ps[lt][:, :],
                        ).then_inc(sems["cpA"])
                    # ph2 even heads: bf16 copies (free banks) + fp8 hi
                    # splits, ordered so cp2v(h4)/cp2v(h6) (ph3 m-set B bank
                    # WARs) are not queued behind hi splits
                    def cp2(h):
                        vector.wait_ge(sems["mm2"], it * H + h + 1)
                        vector.tensor_copy(
                            out=ab_sb[:, h, :], in_=ps[PH2_BANKS[h % 4]][:, :],
                        ).then_inc(sems["cp2v"])

                    def hi(h):
                        vector.tensor_copy(
                            out=a8_sb[:, 0, h, :], in_=ab_sb[:, h, :],
                        ).then_inc(sems["phiV"])

                    cp2(0); hi(0); cp2(2); cp2(4); hi(2); cp2(6); hi(4); hi(6)
                    for h in (0, 2, 4, 6):
                        vector.tensor_sub(
                            a8_sb[:, 1, h, :], ab_sb[:, h, :], a8_sb[:, 0, h, :],
                        ).then_inc(sems["ploV"])
                    for m in (0, 2, 4, 6):
                        vector.wait_ge(sems["mm3"], it * KT + m + 1)
                        if it > 0:
                            vector.wait_ge(sems[f"dmo{m}"], it * 16)
                        vector.tensor_copy(
                            out=o_sb[:, m, :], in_=ps[m][:, :],
                        ).then_inc(sems["cp3v"])

            @block.gpsimd
            def _(gpsimd: bass.BassEngine):
                # zero the PE-warmup tile before anything else
                gpsimd.memset(zdum[:], 0).then_inc(sems["zd"])
                for it in range(iters):
                    # ph2 odd heads: fp8 hi then lo splits (SBUF only)
                    for h in (1, 3, 5, 7):
                        gpsimd.wait_ge(sems["cp2s"], it * 4 + h // 2 + 1)
                        gpsimd.tensor_copy(
                            out=a8_sb[:, 0, h, :], in_=ab_sb[:, h, :],
                        ).then_inc(sems["phiP"])
                    for h in (1, 3, 5, 7):
                        gpsimd.tensor_sub(
                            a8_sb[:, 1, h, :], ab_sb[:, h, :], a8_sb[:, 0, h, :],
                        ).then_inc(sems["ploP"])
                    for m in range(7):
                        gpsimd.wait_ge(cp3_sem(m), cp3_count(m, it))
                        gpsimd.dma_start(
                            out=out[128 * m:128 * m + 128, :],
                            in_=o_sb[:, m, :],
                        ).then_inc(sems[f"dmo{m}"], 16)

                for m in range(KT):
                    gpsimd.wait_ge(sems[f"dmo{m}"], iters * 16)

            @block.scalar
            def _(scalar: bass.BassEngine):
                for it in range(iters):
                    # wave B copies: v[:, lt, 512:1024] from banks [5,6,7,0,1]
                    for lt in range(LT):
                        scalar.wait_ge(sems["mm1"], it * LT + lt + 1)
                        scalar.copy(v_sb[:, lt, 512:1024],
                                    ps[WB[lt]][:, :]).then_inc(sems["cpB"])
                    for h in (1, 3, 5, 7):
                        scalar.wait_ge(sems["mm2"], it * H + h + 1)
                        if it > 0:
                            # ab_sb[h] reuse: prev-iter lo split (Pool) done
                            scalar.wait_ge(sems["ploP"], (it - 1) * 4 + h // 2 + 1)
                        scalar.copy(ab_sb[:, h, :],
                                    ps[PH2_BANKS[h % 4]][:, :]).then_inc(sems["cp2s"])
                    for m in (1, 3, 5, 7):
                        scalar.wait_ge(sems["mm3"], it * KT + m + 1)
                        if it > 0:
                            scalar.wait_ge(sems[f"dmo{m}"], it * 16)
                        scalar.copy(o_sb[:, m, :],
                                    ps[m][:, :]).then_inc(sems["cp3s"])
                        if m == 7:
                            # tail: issue m7's store here, skipping the Pool hop
                            scalar.dma_start(
                                out=out[896:1024, :], in_=o_sb[:, 7, :],
                            ).then_inc(sems["dmo7"], 16)

    return nc


# ---------------- host side ----------------

_GRAPH_CACHE: dict = {}


def get_graph(iters: int = 1, banded: bool = True) -> bass.Bass:
    key = (iters, banded)
    if key not in _GRAPH_CACHE:
        _GRAPH_CACHE[key] = build_graph(iters, banded)
    return _GRAPH_CACHE[key]


class Runner:
    """Compile-once executor for one Bass graph across the 8 cores.

    Mirrors bass2jax.run_bass_via_pjrt but keeps the jitted callable so
    repeated invocations don't re-trace/re-compile.
    """

    def __init__(self, nc: bass.Bass, n_cores: int = N_CORES):
        import jax
        from jax.sharding import Mesh, PartitionSpec
        from jax.experimental.shard_map import shard_map
        from concourse import bass2jax, mybir as _mb

        bass2jax.install_neuronx_cc_hook()
        self.n_cores = n_cores

        partition_name = (nc.partition_id_tensor.name
                          if nc.partition_id_tensor else None)
        in_names, out_names, out_avals, zero_shapes = [], [], [], []
        for alloc in nc.m.functions[0].allocations:
            if not isinstance(alloc, _mb.MemoryLocationSet):
                continue
            name = alloc.memorylocations[0].name
            if alloc.kind == "ExternalInput":
                if name != partition_name:
                    in_names.append(name)
            elif alloc.kind == "ExternalOutput":
                out_names.append(name)
                shape = tuple(alloc.tensor_shape)
                dtype = _mb.dt.np(alloc.dtype)
                out_avals.append(jax.core.ShapedArray(shape, dtype))
                zero_shapes.append((shape, dtype))
        self.in_names = list(in_names)
        self.out_names = out_names
        self.out_avals = out_avals
        self.zero_shapes = zero_shapes
        n_params = len(in_names)
        all_names = in_names + out_names
        if partition_name is not None:
            all_names = all_names + [partition_name]

        def _body(*args):
            operands = list(args)
            if partition_name is not None:
                operands.append(bass2jax.partition_id_tensor())
            outs = bass2jax._bass_exec_p.bind(
                *operands,
                out_avals=tuple(out_avals),
                in_names=tuple(all_names),
                out_names=tuple(out_names),
                lowering_input_output_aliases=(),
                sim_require_finite=True,
                sim_require_nnan=True,
                nc=nc,
            )
            return tuple(outs)

        devices = jax.devices()[:n_cores]
        mesh = Mesh(np.asarray(devices), ("core",))
        self._mesh = mesh
        n_outs = len(out_names)
        self._fn = jax.jit(
            shard_map(_body, mesh=mesh,
                      in_specs=(PartitionSpec("core"),) * (n_params + n_outs),
                      out_specs=(PartitionSpec("core"),) * n_outs,
                      check_rep=False),
            donate_argnums=tuple(range(n_params, n_params + n_outs)),
            keep_unused=True,
        )

    def stage(self, in_maps):
        """device_put the concatenated inputs once; returns device arrays."""
        import jax
        concat_in = [
            np.concatenate([np.asarray(m[name]) for m in in_maps], axis=0)
            for name in self.in_names
        ]
        return [jax.device_put(a) for a in concat_in]

    def make_zeros(self):
        if not hasattr(self, "_zeros_fn"):
            import jax
            import jax.numpy as jnp
            from jax.sharding import NamedSharding, PartitionSpec
            shardings = tuple(
                NamedSharding(self._mesh, PartitionSpec("core"))
                for _ in self.zero_shapes)
            shapes = [((self.n_cores * s[0], *s[1:]), d)
                      for s, d in self.zero_shapes]

            def _mk():
                return tuple(jnp.zeros(sh, dt) for sh, dt in shapes)

            self._zeros_fn = jax.jit(_mk, out_shardings=shardings)
        return list(self._zeros_fn())

    def run_staged(self, dev_in, dev_zeros):
        return self._fn(*dev_in, *dev_zeros)

    def __call__(self, in_maps):
        out_arrs = self._fn(*self.stage(in_maps), *self.make_zeros())
        return [
            {name: np.asarray(out_arrs[i]).reshape(
                self.n_cores, *self.out_avals[i].shape)[c]
             for i, name in enumerate(self.out_names)}
            for c in range(self.n_cores)
        ]


_RUNNER_CACHE: dict = {}


def get_runner(iters: int = 1) -> "Runner":
    if iters not in _RUNNER_CACHE:
        _RUNNER_CACHE[iters] = Runner(get_graph(iters))
    return _RUNNER_CACHE[iters]


def _split8(a: np.ndarray) -> np.ndarray:
    """[2, ...] stack of (hi, lo) fp8e4 halves of a float32 array."""
    hi = a.astype(np.float32).astype(NPF8)
    lo = (a.astype(np.float32) - hi.astype(np.float32)).astype(NPF8)
    return np.stack([hi, lo])


def make_in_maps(values: np.ndarray, input_weights: np.ndarray,
                 output_weight: np.ndarray) -> list:
    w1s = _split8(np.ascontiguousarray(input_weights.T) * S_W)
    w2s = _split8(np.ascontiguousarray(output_weight.T) * S_W)
    tpt = gauss_toeplitz_table()
    in_maps = []
    for core in range(N_CORES):
        b, c = divmod(core, 4)
        lo, hi = c * CHUNK - HALO_L, c * CHUNK + CHUNK + HALO_R
        src_lo, src_hi = max(lo, 0), min(hi, L)
        xt_pad = np.zeros((D, LPAD), dtype=np.float32)
        xt_pad[:, src_lo - lo:src_hi - lo] = values[b, src_lo:src_hi, :].T
        in_maps.append({"xt": _split8(xt_pad), "w1": w1s, "w2": w2s, "tp": tpt})
    return in_maps


def assemble(results: list) -> np.ndarray:
    out = np.empty((B, L, D), dtype=np.float32)
    for core in range(N_CORES):
        b, c = divmod(core, 4)
        out[b, c * CHUNK:(c + 1) * CHUNK, :] = \
            results[core]["out"].T.astype(np.float32) * OUT_DESCALE
    return out


def kernel(values: np.ndarray, input_weights: np.ndarray,
           output_weight: np.ndarray) -> np.ndarray:
    in_maps = make_in_maps(values, input_weights, output_weight)
    try:
        return assemble(get_runner(1)(in_maps))
    except Exception:
        # fallback: canonical SPMD path (re-traces per call but always works)
        res = run_bass_kernel_spmd(get_graph(1), in_maps,
                                   core_ids=list(range(N_CORES)))
        return assemble(res.results)


# revision 35
# speedup vs baseline: 1.1673x; 1.0032x over previous
"""Trainium2 Bass kernel for nn_Attention (Gaussian banded attention).

Math (reference):
    v = values @ input_weights.T                      # [B,L,D]
    probs[h,q,k] = N(k - q - off_h; std_h)            # Gaussian, depends on k-q only
    attended[b,h,q,:] = sum_k probs[h,q,k] v[b,k,h*pd:(h+1)*pd]
    out = attended_merged @ output_weight.T           # [B,L,D]

Structural facts exploited:
  - probs is banded Toeplitz per head (6-sigma truncation) -> attention is a
    narrow depthwise conv along L, done as windowed matmuls vs a 128x1024
    Toeplitz table. Batch x L sharding is embarrassingly parallel with a
    56/40-row input halo (8 cores = 2 batches x 4 chunks of 512 rows).
  - The two dense 1024x1024 projections dominate PE time. They run as
    fp8e4(DoubleRow) matmuls: each instruction contracts 2x128 rows at
    0.5 cycles/row = 4x bf16 throughput. Full bf16-level precision is kept
    by splitting each operand x = hi + lo (both fp8) and accumulating
    three of the four cross terms in PSUM (hi*hi + lo*hi + hi*lo); the
    dropped lo*lo term is ~0.1% relative. Net projection cost: 6/8 of bf16.
  - Weights are pre-scaled by 256 (fp8e4 subnormal cutoff is 2^-6; raw
    weights have sigma 0.02), the Gaussian table by 4/256, and the host
    divides the output by 1024. All scales are powers of two (exact).

Phase structure per iteration (PE program order):
  warmup: discarded matmuls during the first DMA latency window;
  wave A: v[:, 0:512]   fp8 DoubleRow, banks 0-4 (bank=lt), kp-outer with
          passes HH,LH interleaved per kp and HL trailing (DMA streaming);
  wave B: v[:, 512:1024] banks [5,6,7,0,1], lt-outer, 12 matmuls/group;
  ph2:    attendedT per head, bf16 banded Toeplitz windows, banks [2,3,4,5];
          PSUM->SBUF copy to bf16 (DVE evens / Act odds) releases banks;
          fp8 hi/lo split runs in SBUF on DVE (evens) and Pool (odds);
  ph3:    outT = W2' @ attendedT, all 8 banks (bank=m); hi-consuming passes
          (HH then HL) kp-outer first, lo-consuming pass (LH) m-outer last
          so the fp8 splits hide behind ~7us of matmuls.
"""

import math
from contextlib import ExitStack

import numpy as np
import ml_dtypes

import concourse.bass as bass
from concourse import mybir
from concourse.bass_utils import run_bass_kernel_spmd

# ---- NEFF disk cache (keyed by BIR hash) to avoid recompiling identical
# graphs in fresh processes ----
import hashlib
import os
import shutil

_NEFF_CACHE_DIR = os.environ.get("NEFF_CACHE_DIR", "/root/neff_cache")


def _install_neff_cache():
    import concourse.bass_utils as _bu
    import concourse.bass2jax as _b2j
    if getattr(_bu, "_neff_cache_installed", False):
        return
    orig = _bu.compile_bir_kernel

    def cached(bir_json, tmpdir, neff_name="file.neff"):
        cpath = None
        try:
            os.makedirs(_NEFF_CACHE_DIR, exist_ok=True)
            key = hashlib.sha256(bir_json).hexdigest()[:32]
            cpath = os.path.join(_NEFF_CACHE_DIR, f"{key}.neff")
            dst = os.path.join(tmpdir, neff_name)
            if os.path.exists(cpath):
                shutil.copy(cpath, dst)
                return dst
        except OSError:
            cpath = None  # cache unusable; plain compile below
        path = orig(bir_json, tmpdir, neff_name)
        if cpath is not None:
            try:
                shutil.copy(path, cpath)
            except OSError:
                pass
        return path

    _bu.compile_bir_kernel = cached
    _b2j.compile_bir_kernel = cached
    _bu._neff_cache_installed = True


_install_neff_cache()

# ---------------- problem constants (hardcoded per spec) ----------------
B, L, D = 2, 2048, 1024
H, PD = 8, 128
ATTN_STD = np.array([1.0, 2.0, 4.0, 8.0, 1.0, 2.0, 4.0, 8.0], dtype=np.float64)
ATTN_OFFSET = np.array([-1.0, -2.0, -4.0, -8.0, -1.0, -2.0, -4.0, -8.0], dtype=np.float64)

N_CORES = 8
CHUNK = 512            # output rows per core
HALO_L, HALO_R = 56, 40
LPAD = 640             # 56 + 512 + 40 = 608, padded to 5*128
LT = 5                 # l-tiles of v (640 / 128)
KT = 8                 # d tiles (1024 / 128)
KP = 4                 # DoubleRow k-pairs (1024 / 256)
NQ = CHUNK             # query columns per core

BF16 = mybir.dt.bfloat16
F8 = mybir.dt.float8e4
F32 = mybir.dt.float32
DR = mybir.MatmulPerfMode.DoubleRow
NPF8 = ml_dtypes.float8_e4m3
NPBF = ml_dtypes.bfloat16

# power-of-two scales: weights *256 (clear fp8 subnormals), Gaussian table
# *4/256 (S1 cancel + attended into fp8 sweet spot), host output /(4*256)
S_W = 256.0
S_A = 4.0
OUT_DESCALE = 1.0 / (S_A * S_W)

TP0, TPW = 408, 256        # banded Toeplitz window (512B rows: no DMA penalty)

WB = [5, 6, 7, 0, 1]       # wave B bank per lt
PH2_BANKS = [2, 3, 4, 5, 0, 1, 2, 3]   # ph2 bank per head (6 banks used)

# p-state warmup: discarded matmul sizes (rows), burned during the first
# DMA latency window; tuned so the PE reaches the first wave-A wait just
# after data-readiness
WARMUP = (256, 256, 256, 184)


def gauss_toeplitz_table() -> np.ndarray:
    """tp[h, r, m] = g_h(r - (m - 512) - 56) * S_A/S_W, shape [H,128,1024] bf16.

    For v-tile t (rows k' = 128t + r of padded-local v) the attention rhs is
    tp[h][:, 512-128t : 1024-128t] so that rhs[r, q'] = g_h(128t + r - q' - 56),
    which is probs[h, q, k].T in padded-local coordinates.
    """
    r = np.arange(128, dtype=np.float64)[:, None]
    m = np.arange(1024, dtype=np.float64)[None, :]
    delta = r - (m - 512.0) - 56.0  # = k - q
    tables = []
    for h in range(H):
        std, off = ATTN_STD[h], ATTN_OFFSET[h]
        z = (delta - off) / std
        g = np.exp(-0.5 * z * z) / (std * math.sqrt(2.0 * math.pi))
        g[np.abs(z) > 6.0] = 0.0
        tables.append(g * (S_A / S_W))
    full = np.stack(tables).astype(NPBF)          # [H, 128, 1024]
    return np.ascontiguousarray(full[:, :, TP0:TP0 + TPW])


def attn_windows(h: int):
    """Static (t, j0, j1) list: nonzero q-column window of v-tile t for head h,
    8-aligned. Coverage of [0,512) is guaranteed (window width > 128)."""
    std, off = int(ATTN_STD[h]), int(ATTN_OFFSET[h])
    wlo = -56 - off - 6 * std
    whi = 71 - off + 6 * std
    res = []
    for t in range(LT):
        j0 = max(0, 128 * t + wlo)
        j1 = min(NQ, 128 * t + whi + 1)
        if j0 >= j1:
            continue
        j0 = (j0 // 8) * 8
        j1 = min(NQ, ((j1 + 7) // 8) * 8)
        res.append((t, j0, j1))
    return res


def build_graph(iters: int = 1, banded: bool = True) -> bass.Bass:
    """One SPMD core program. iters>1 repeats the whole kernel (including
    DMAs) with monotonically increasing semaphore thresholds, for timing."""
    nc = bass.Bass()

    xt = nc.declare_dram_parameter("xt", [2, D, LPAD], F8, isOutput=False)
    w1 = nc.declare_dram_parameter("w1", [2, D, D], F8, isOutput=False)
    w2 = nc.declare_dram_parameter("w2", [2, D, D], F8, isOutput=False)
    tp = nc.declare_dram_parameter("tp", [H, 128, TPW], BF16, isOutput=False)
    out = nc.declare_dram_parameter("out", [D, NQ], BF16, isOutput=True)

    xt_r = xt[:].rearrange("s (o p) f -> p s o f", p=128)   # [128, 2, 8, 640]
    w1_r = w1[:].rearrange("s (o p) f -> p s o f", p=128)   # [128, 2, 8, 1024]
    w2_r = w2[:].rearrange("s (o p) f -> p s o f", p=128)   # [128, 2, 8, 1024]
    tp_r = tp[:].rearrange("h p f -> p h f")                # [128, 8, 256]

    with ExitStack() as ctx:
        e = ctx.enter_context
        xt_sb = e(nc.sbuf_tensor("xt_sb", [128, 2, 2, KT, LPAD], F8))
        w1_sb = e(nc.sbuf_tensor("w1_sb", [128, 2, 2, KT, D], F8))
        w2_sb = e(nc.sbuf_tensor("w2_sb", [128, 2, KT, D], F8))
        tp_sb = e(nc.sbuf_tensor("tp_sb", [128, H, TPW], BF16))
        tp_src = tp_r[:, :, :]
        v_sb = e(nc.sbuf_tensor("v_sb", [128, LT, D], BF16))
        ab_sb = e(nc.sbuf_tensor("ab_sb", [128, H, NQ], BF16))   # attended bf16
        a8_sb = e(nc.sbuf_tensor("a8_sb", [128, 2, KT, NQ], F8))  # hi/lo fp8
        o_sb = e(nc.sbuf_tensor("o_sb", [128, KT, NQ], BF16))
        zdum = e(nc.sbuf_tensor("zdum", [128, 384], BF16))
        ps = [e(nc.psum_tensor(f"ps{i}", [128, 512], F32)) for i in range(8)]

        sem_names = (["zd", "mmA", "mm1", "mm2", "mm3", "tp_d",
                      "cpA", "cpB", "cp2v", "cp2s", "cp3v", "cp3s",
                      "phiV", "phiP", "ploV", "ploP", "w2h", "w2l"]
                     + [f"{n}b{p}" for n in ("xh01", "xh23", "xl01", "xl23",
                                             "wah01", "wah23", "wal01", "wal23",
                                             "wbh", "wbl") for p in (0, 1)]
                     + [f"dmo{m}" for m in range(KT)])
        sems = {n: e(nc.semaphore(n)) for n in sem_names}

        def cp2_sem(h):
            return sems["cp2v" if h % 2 == 0 else "cp2s"]

        def cp2_count(h, it):
            return it * 4 + h // 2 + 1

        def cp3_sem(m):
            return sems["cp3v" if m % 2 == 0 else "cp3s"]

        def cp3_count(m, it):
            return it * 4 + m // 2 + 1

        with nc.Block() as block:

            @block.sync
            def _(sync: bass.BassEngine):
                for it in range(iters):
                    buf = it % 2
                    if it > 1:
                        # xt/w1 buffer reuse: wave B HL (last reader) of it-2
                        sync.wait_ge(sems["mm1"], (it - 1) * LT)

                    def dma(dst, src, sem):
                        sync.dma_start(out=dst, in_=src).then_inc(sems[sem], 16)

                    # coarse chunks, issued in wave A consumption order
                    # (the shared HWDGE generator costs 625ns per issue)
                    dma(xt_sb[:, buf, 0, 0:4, :], xt_r[:, 0, 0:4, :],
                        f"xh01b{buf}")
                    dma(w1_sb[:, buf, 0, 0:4, 0:512], w1_r[:, 0, 0:4, 0:512],
                        f"wah01b{buf}")
                    dma(xt_sb[:, buf, 0, 4:8, :], xt_r[:, 0, 4:8, :],
                        f"xh23b{buf}")
                    dma(w1_sb[:, buf, 0, 4:8, 0:512], w1_r[:, 0, 4:8, 0:512],
                        f"wah23b{buf}")
                    dma(xt_sb[:, buf, 1, 0:4, :], xt_r[:, 1, 0:4, :],
                        f"xl01b{buf}")
                    dma(xt_sb[:, buf, 1, 4:8, :], xt_r[:, 1, 4:8, :],
                        f"xl23b{buf}")
                    dma(w1_sb[:, buf, 1, 0:4, 0:512], w1_r[:, 1, 0:4, 0:512],
                        f"wal01b{buf}")
                    dma(w1_sb[:, buf, 1, 4:8, 0:512], w1_r[:, 1, 4:8, 0:512],
                        f"wal23b{buf}")
                    dma(w1_sb[:, buf, 0, :, 512:1024], w1_r[:, 0, :, 512:1024],
                        f"wbhb{buf}")
                    dma(w1_sb[:, buf, 1, :, 512:1024], w1_r[:, 1, :, 512:1024],
                        f"wblb{buf}")
                    if it == 0:
                        sync.dma_start(out=tp_sb[:], in_=tp_src).then_inc(
                            sems["tp_d"], 16)
                    if it > 0:
                        sync.wait_ge(sems["mm3"], it * KT)
                    dma(w2_sb[:, 0, :, :], w2_r[:, 0, :, :], "w2h")
                    dma(w2_sb[:, 1, :, :], w2_r[:, 1, :, :], "w2l")

            @block.tensor
            def _(tensor: bass.BassEngine):
                # HAM/p-state warmup: discarded matmuls into bank 0 while the
                # first input DMAs are in flight
                tensor.wait_ge(sems["zd"], 1)
                for wn in WARMUP:
                    tensor.matmul(ps[0][:, 0:wn], zdum[:, 0:128],
                                  zdum[:, 128:128 + wn], start=True, stop=True)

                def dr_mm(bank, lhsT, rhs, start, stop):
                    return tensor.matmul(bank[:, :], lhsT, rhs,
                                         start=start, stop=stop, perf_mode=DR)

                for it in range(iters):
                    buf = it % 2
                    nth = (it // 2 + 1) * 16  # per-parity DMA count

                    # ---- wave A: v[:, 0:512], banks 0-4 (bank = lt) ----
                    # kp-outer; passes HH (x_hi*w_hi) and LH (x_lo*w_hi)
                    # interleaved per kp to match DMA arrival order; HL
                    # (x_hi*w_lo) trails.
                    for t in range(KP):
                        if t == 0:
                            tensor.wait_ge(sems[f"xh01b{buf}"], nth)
                            tensor.wait_ge(sems[f"wah01b{buf}"], nth)
                        elif t == 2:
                            tensor.wait_ge(sems[f"xh23b{buf}"], nth)
                            tensor.wait_ge(sems[f"wah23b{buf}"], nth)
                        for lt in range(LT):
                            if t == 0 and it > 0:
                                # bank lt <- ph3 m=lt copy of prev iter
                                tensor.wait_ge(cp3_sem(lt), cp3_count(lt, it - 1))
                            dr_mm(ps[lt],
                                  xt_sb[:, buf, 0, 2 * t:2 * t + 2,
                                        128 * lt:128 * lt + 128],
                                  w1_sb[:, buf, 0, 2 * t:2 * t + 2, 0:512],
                                  start=(t == 0), stop=False)
                    for t in range(KP):
                        if t == 0:
                            tensor.wait_ge(sems[f"xl01b{buf}"], nth)
                        elif t == 2:
                            tensor.wait_ge(sems[f"xl23b{buf}"], nth)
                        for lt in range(LT):
                            dr_mm(ps[lt],
                                  xt_sb[:, buf, 1, 2 * t:2 * t + 2,
                                        128 * lt:128 * lt + 128],
                                  w1_sb[:, buf, 0, 2 * t:2 * t + 2, 0:512],
                                  start=False, stop=False)
                    for t in range(KP):
                        if t == 0:
                            tensor.wait_ge(sems[f"wal01b{buf}"], nth)
                        elif t == 2:
                            tensor.wait_ge(sems[f"wal23b{buf}"], nth)
                        for lt in range(LT):
                            mm = dr_mm(ps[lt],
                                       xt_sb[:, buf, 0, 2 * t:2 * t + 2,
                                             128 * lt:128 * lt + 128],
                                       w1_sb[:, buf, 1, 2 * t:2 * t + 2, 0:512],
                                       start=False, stop=(t == KP - 1))
                            if t == KP - 1:
                                mm.then_inc(sems["mmA"])

                    # ---- wave B: v[:, 512:1024], banks [5,6,7,0,1], lt-outer --
                    for lt in range(LT):
                        bank = ps[WB[lt]]
                        if it > 0 and lt == 0:
                            tensor.wait_ge(cp3_sem(5), cp3_count(5, it - 1))
                        elif it > 0 and lt == 1:
                            tensor.wait_ge(cp3_sem(6), cp3_count(6, it - 1))
                        elif it > 0 and lt == 2:
                            tensor.wait_ge(cp3_sem(7), cp3_count(7, it - 1))
                        elif lt == 3:
                            tensor.wait_ge(sems["cpA"], it * LT + 1)
                        elif lt == 4:
                            tensor.wait_ge(sems["cpA"], it * LT + 2)
                        for pas in range(3):  # HH, LH, HL
                            sx = 1 if pas == 1 else 0
                            sw = 1 if pas == 2 else 0
                            for t in range(KP):
                                if lt == 0 and pas == 0 and t == 0:
                                    tensor.wait_ge(sems[f"wbhb{buf}"], nth)
                                if lt == 0 and pas == 2 and t == 0:
                                    tensor.wait_ge(sems[f"wblb{buf}"], nth)
                                mm = dr_mm(
                                    bank,
                                    xt_sb[:, buf, sx, 2 * t:2 * t + 2,
                                          128 * lt:128 * lt + 128],
                                    w1_sb[:, buf, sw, 2 * t:2 * t + 2, 512:1024],
                                    start=(pas == 0 and t == 0),
                                    stop=(pas == 2 and t == KP - 1))
                                if pas == 2 and t == KP - 1:
                                    mm.then_inc(sems["mm1"])

                    # ---- phase 2: attendedT per head (bf16 banded) ----
                    if it == 0:
                        tensor.wait_ge(sems["tp_d"], 16)
                    for h in range(H):
                        bank = ps[PH2_BANKS[h]]
                        # bank WAR: banks 2,3,4 <- wave A lt=2,3,4 copies;
                        # bank 5 <- wave B lt=0 copy; banks 0,1 <- wave B
                        # lt=3,4 copies; h>=6 <- ph2 head h-6 copy
                        if h == 0:
                            tensor.wait_ge(sems["cpA"], it * LT + 3)
                        elif h == 1:
                            tensor.wait_ge(sems["cpA"], it * LT + 4)
                        elif h == 2:
                            tensor.wait_ge(sems["cpA"], it * LT + 5)
                        elif h == 3:
                            tensor.wait_ge(sems["cpB"], it * LT + 1)
                        elif h == 4:
                            tensor.wait_ge(sems["cpB"], it * LT + 4)
                        elif h == 5:
                            tensor.wait_ge(sems["cpB"], it * LT + 5)
                        else:
                            tensor.wait_ge(cp2_sem(h - 6), cp2_count(h - 6, it))
                        windows = attn_windows(h)
                        for wi, (t, j0, j1) in enumerate(windows):
                            if h // 4 == 0:
                                tensor.wait_ge(sems["cpA"], it * LT + t + 1)
                            else:
                                tensor.wait_ge(sems["cpB"], it * LT + t + 1)
                            c0 = 512 - 128 * t + j0 - TP0
                            c1 = 512 - 128 * t + j1 - TP0
                            mm = tensor.matmul(
                                bank[:, j0:j1],
                                v_sb[:, t, 128 * h:128 * h + 128],
                                tp_sb[:, h, c0:c1],
                                start=(wi == 0), stop=(wi == len(windows) - 1),
                            )
                            if wi == len(windows) - 1:
                                mm.then_inc(sems["mm2"])

                    # ---- phase 3: outT = W2' @ attendedT, banks 0-7 ----
                    # stage 1 (hi-consuming) is fully kp-outer so the at8-hi
                    # for pair t is needed only ~1.7us*t after ph3 starts,
                    # matching the pace the splits come off the copy engines.
                    # Per kp: HH/HL over m-set A (banks from wave B copies),
                    # then over m-set B (banks from ph2 copies, later).
                    # stage 2 (LH, lo-consuming) is m-outer and closes groups.
                    MSET_A = (6, 7, 4, 5)
                    MSET_B = (0, 1, 2, 3)
                    for t in range(KP):
                        tensor.wait_ge(sems["phiV"], it * 4 + t + 1)
                        tensor.wait_ge(sems["phiP"], it * 4 + t + 1)
                        if t == 0:
                            tensor.wait_ge(sems["w2h"], (it + 1) * 16)
                        for si, mset in enumerate((MSET_A, MSET_B)):
                            for pas in range(2):  # HH, HL
                                if t == 0 and si == 0 and pas == 1:
                                    tensor.wait_ge(sems["w2l"], (it + 1) * 16)
                                for m in mset:
                                    if t == 0 and pas == 0:
                                        # bank WAR: last ph2 user of bank m
                                        if m == 6:
                                            tensor.wait_ge(sems["cpB"],
                                                           it * LT + 2)
                                        elif m == 7:
                                            tensor.wait_ge(sems["cpB"],
                                                           it * LT + 3)
                                        else:
                                            hh = {0: 4, 1: 5, 2: 6, 3: 7,
                                                  4: 2, 5: 3}[m]
                                            tensor.wait_ge(cp2_sem(hh),
                                                           cp2_count(hh, it))
                                    dr_mm(ps[m],
                                          w2_sb[:, pas, 2 * t:2 * t + 2,
                                                128 * m:128 * m + 128],
                                          a8_sb[:, 0, 2 * t:2 * t + 2, :],
                                          start=(t == 0 and pas == 0),
                                          stop=False)
                    for m in range(KT):  # LH pass, closes groups
                        for t in range(KP):
                            if m == 0:
                                tensor.wait_ge(sems["ploV"], it * 4 + t + 1)
                                tensor.wait_ge(sems["ploP"], it * 4 + t + 1)
                            mm = dr_mm(ps[m],
                                       w2_sb[:, 0, 2 * t:2 * t + 2,
                                             128 * m:128 * m + 128],
                                       a8_sb[:, 1, 2 * t:2 * t + 2, :],
                                       start=False, stop=(t == KP - 1))
                            if t == KP - 1:
                                mm.then_inc(sems["mm3"])

            @block.vector
            def _(vector: bass.BassEngine):
                for it in range(iters):
                    # wave A copies: v[:, lt, 0:512] from banks 0-4
                    for lt in range(LT):
                        vector.wait_ge(sems["mmA"], it * LT + lt + 1)
                        vector.tensor_copy(
                            out=v_sb[:, lt, 0:512], in_=ps[lt][:, :],
                        ).then_inc(sems["cpA"])
                    # ph2 even heads: bf16 copies (free banks) + fp8 hi
                    # splits, ordered so cp2v(h4)/cp2v(h6) (ph3 m-set B bank
                    # WARs) are not queued behind hi splits
                    def cp2(h):
                        vector.wait_ge(sems["mm2"], it * H + h + 1)
                        vector.tensor_copy(
                            out=ab_sb[:, h, :], in_=ps[PH2_BANKS[h]][:, :],
                        ).then_inc(sems["cp2v"])

                    def hi(h):
                        vector.tensor_copy(
                            out=a8_sb[:, 0, h, :], in_=ab_sb[:, h, :],
                        ).then_inc(sems["phiV"])

                    cp2(0); hi(0); cp2(2); cp2(4); hi(2); cp2(6); hi(4); hi(6)
                    for h in (0, 2, 4, 6):
                        vector.tensor_sub(
                            a8_sb[:, 1, h, :], ab_sb[:, h, :], a8_sb[:, 0, h, :],
                        ).then_inc(sems["ploV"])
                    for m in (0, 2, 4, 6):
                        vector.wait_ge(sems["mm3"], it * KT + m + 1)
                        if it > 0:
                            vector.wait_ge(sems[f"dmo{m}"], it * 16)
                        vector.tensor_copy(
                            out=o_sb[:, m, :], in_=ps[m][:, :],
                        ).then_inc(sems["cp3v"])

            @block.gpsimd
            def _(gpsimd: bass.BassEngine):
                # zero the PE-warmup tile before anything else
                gpsimd.memset(zdum[:], 0).then_inc(sems["zd"])
                for it in range(iters):
                    # ph2 odd heads: fp8 hi then lo splits (SBUF only)
                    for h in (1, 3, 5, 7):
                        gpsimd.wait_ge(sems["cp2s"], it * 4 + h // 2 + 1)
                        gpsimd.tensor_copy(
                            out=a8_sb[:, 0, h, :], in_=ab_sb[:, h, :],
                        ).then_inc(sems["phiP"])
                    for h in (1, 3, 5, 7):
                        gpsimd.tensor_sub(
                            a8_sb[:, 1, h, :], ab_sb[:, h, :], a8_sb[:, 0, h, :],
                        ).then_inc(sems["ploP"])
                    for m in range(7):
                        gpsimd.wait_ge(cp3_sem(m), cp3_count(m, it))
                        gpsimd.dma_start(
                            out=out[128 * m:128 * m + 128, :],
                            in_=o_sb[:, m, :],
                        ).then_inc(sems[f"dmo{m}"], 16)

                for m in range(KT):
                    gpsimd.wait_ge(sems[f"dmo{m}"], iters * 16)

            @block.scalar
            def _(scalar: bass.BassEngine):
                for it in range(iters):
                    # wave B copies: v[:, lt, 512:1024] from banks [5,6,7,0,1]
                    for lt in range(LT):
                        scalar.wait_ge(sems["mm1"], it * LT + lt + 1)
                        scalar.copy(v_sb[:, lt, 512:1024],
                                    ps[WB[lt]][:, :]).then_inc(sems["cpB"])
                    for h in (1, 3, 5, 7):
                        scalar.wait_ge(sems["mm2"], it * H + h + 1)
                        if it > 0:
                            # ab_sb[h] reuse: prev-iter lo split (Pool) done
                            scalar.wait_ge(sems["ploP"], (it - 1) * 4 + h // 2 + 1)
                        scalar.copy(ab_sb[:, h, :],
                                    ps[PH2_BANKS[h]][:, :]).then_inc(sems["cp2s"])
                    for m in (1, 3, 5, 7):
                        scalar.wait_ge(sems["mm3"], it * KT + m + 1)
                        if it > 0:
                            scalar.wait_ge(sems[f"dmo{m}"], it * 16)
                        scalar.copy(o_sb[:, m, :],
                                    ps[m][:, :]).then_inc(sems["cp3s"])
                        if m == 7:
                            # tail: issue m7's store here, skipping the Pool hop
                            scalar.dma_start(
                                out=out[896:1024, :], in_=o_sb[:, 7, :],
                            ).then_inc(sems["dmo7"], 16)

    return nc


# ---------------- host side ----------------

_GRAPH_CACHE: dict = {}


def get_graph(iters: int = 1, banded: bool = True) -> bass.Bass:
    key = (iters, banded)
    if key not in _GRAPH_CACHE:
        _GRAPH_CACHE[key] = build_graph(iters, banded)
    return _GRAPH_CACHE[key]


class Runner:
    """Compile-once executor for one Bass graph across the 8 cores.

    Mirrors bass2jax.run_bass_via_pjrt but keeps the jitted callable so
    repeated invocations don't re-trace/re-compile.
    """

    def __init__(self, nc: bass.Bass, n_cores: int = N_CORES):
        import jax
        from jax.sharding import Mesh, PartitionSpec
        from jax.experimental.shard_map import shard_map
        from concourse import bass2jax, mybir as _mb

        bass2jax.install_neuronx_cc_hook()
        self.n_cores = n_cores

        partition_name = (nc.partition_id_tensor.name
                          if nc.partition_id_tensor else None)
        in_names, out_names, out_avals, zero_shapes = [], [], [], []
        for alloc in nc.m.functions[0].allocations:
            if not isinstance(alloc, _mb.MemoryLocationSet):
                continue
            name = alloc.memorylocations[0].name
            if alloc.kind == "ExternalInput":
                if name != partition_name:
                    in_names.append(name)
            elif alloc.kind == "ExternalOutput":
                out_names.append(name)
                shape = tuple(alloc.tensor_shape)
                dtype = _mb.dt.np(alloc.dtype)
                out_avals.append(jax.core.ShapedArray(shape, dtype))
                zero_shapes.append((shape, dtype))
        self.in_names = list(in_names)
        self.out_names = out_names
        self.out_avals = out_avals
        self.zero_shapes = zero_shapes
        n_params = len(in_names)
        all_names = in_names + out_names
        if partition_name is not None:
            all_names = all_names + [partition_name]

        def _body(*args):
            operands = list(args)
            if partition_name is not None:
                operands.append(bass2jax.partition_id_tensor())
            outs = bass2jax._bass_exec_p.bind(
                *operands,
                out_avals=tuple(out_avals),
                in_names=tuple(all_names),
                out_names=tuple(out_names),
                lowering_input_output_aliases=(),
                sim_require_finite=True,
                sim_require_nnan=True,
                nc=nc,
            )
            return tuple(outs)

        devices = jax.devices()[:n_cores]
        mesh = Mesh(np.asarray(devices), ("core",))
        self._mesh = mesh
        n_outs = len(out_names)
        self._fn = jax.jit(
            shard_map(_body, mesh=mesh,
                      in_specs=(PartitionSpec("core"),) * (n_params + n_outs),
                      out_specs=(PartitionSpec("core"),) * n_outs,
                      check_rep=False),
            donate_argnums=tuple(range(n_params, n_params + n_outs)),
            keep_unused=True,
        )

    def stage(self, in_maps):
        """device_put the concatenated inputs once; returns device arrays."""
        import jax
        concat_in = [
            np.concatenate([np.asarray(m[name]) for m in in_maps], axis=0)
            for name in self.in_names
        ]
        return [jax.device_put(a) for a in concat_in]

    def make_zeros(self):
        if not hasattr(self, "_zeros_fn"):
            import jax
            import jax.numpy as jnp
            from jax.sharding import NamedSharding, PartitionSpec
            shardings = tuple(
                NamedSharding(self._mesh, PartitionSpec("core"))
                for _ in self.zero_shapes)
            shapes = [((self.n_cores * s[0], *s[1:]), d)
                      for s, d in self.zero_shapes]

            def _mk():
                return tuple(jnp.zeros(sh, dt) for sh, dt in shapes)

            self._zeros_fn = jax.jit(_mk, out_shardings=shardings)
        return list(self._zeros_fn())

    def run_staged(self, dev_in, dev_zeros):
        return self._fn(*dev_in, *dev_zeros)

    def __call__(self, in_maps):
        out_arrs = self._fn(*self.stage(in_maps), *self.make_zeros())
        return [
            {name: np.asarray(out_arrs[i]).reshape(
                self.n_cores, *self.out_avals[i].shape)[c]
             for i, name in enumerate(self.out_names)}
            for c in range(self.n_cores)
        ]


_RUNNER_CACHE: dict = {}


def get_runner(iters: int = 1) -> "Runner":
    if iters not in _RUNNER_CACHE:
        _RUNNER_CACHE[iters] = Runner(get_graph(iters))
    return _RUNNER_CACHE[iters]


def _split8(a: np.ndarray) -> np.ndarray:
    """[2, ...] stack of (hi, lo) fp8e4 halves of a float32 array."""
    hi = a.astype(np.float32).astype(NPF8)
    lo = (a.astype(np.float32) - hi.astype(np.float32)).astype(NPF8)
    return np.stack([hi, lo])


def make_in_maps(values: np.ndarray, input_weights: np.ndarray,
                 output_weight: np.ndarray) -> list:
    w1s = _split8(np.ascontiguousarray(input_weights.T) * S_W)
    w2s = _split8(np.ascontiguousarray(output_weight.T) * S_W)
    tpt = gauss_toeplitz_table()
    in_maps = []
    for core in range(N_CORES):
        b, c = divmod(core, 4)
        lo, hi = c * CHUNK - HALO_L, c * CHUNK + CHUNK + HALO_R
        src_lo, src_hi = max(lo, 0), min(hi, L)
        xt_pad = np.zeros((D, LPAD), dtype=np.float32)
        xt_pad[:, src_lo - lo:src_hi - lo] = values[b, src_lo:src_hi, :].T
        in_maps.append({"xt": _split8(xt_pad), "w1": w1s, "w2": w2s, "tp": tpt})
    return in_maps


def assemble(results: list) -> np.ndarray:
    out = np.empty((B, L, D), dtype=np.float32)
    for core in range(N_CORES):
        b, c = divmod(core, 4)
        out[b, c * CHUNK:(c + 1) * CHUNK, :] = \
            results[core]["out"].T.astype(np.float32) * OUT_DESCALE
    return out


def kernel(values: np.ndarray, input_weights: np.ndarray,
           output_weight: np.ndarray) -> np.ndarray:
    in_maps = make_in_maps(values, input_weights, output_weight)
    try:
        return assemble(get_runner(1)(in_maps))
    except Exception:
        # fallback: canonical SPMD path (re-traces per call but always works)
        res = run_bass_kernel_spmd(get_graph(1), in_maps,
                                   core_ids=list(range(N_CORES)))
        return assemble(res.results)


# revision 39
# speedup vs baseline: 1.1733x; 1.0051x over previous
"""Trainium2 Bass kernel for nn_Attention (Gaussian banded attention).

Math (reference):
    v = values @ input_weights.T                      # [B,L,D]
    probs[h,q,k] = N(k - q - off_h; std_h)            # Gaussian, depends on k-q only
    attended[b,h,q,:] = sum_k probs[h,q,k] v[b,k,h*pd:(h+1)*pd]
    out = attended_merged @ output_weight.T           # [B,L,D]

Structural facts exploited:
  - probs is banded Toeplitz per head (6-sigma truncation) -> attention is a
    narrow depthwise conv along L, done as windowed matmuls vs a 128x1024
    Toeplitz table. Batch x L sharding is embarrassingly parallel with a
    56/40-row input halo (8 cores = 2 batches x 4 chunks of 512 rows).
  - The two dense 1024x1024 projections dominate PE time. They run as
    fp8e4(DoubleRow) matmuls: each instruction contracts 2x128 rows at
    0.5 cycles/row = 4x bf16 throughput. Full bf16-level precision is kept
    by splitting each operand x = hi + lo (both fp8) and accumulating
    three of the four cross terms in PSUM (hi*hi + lo*hi + hi*lo); the
    dropped lo*lo term is ~0.1% relative. Net projection cost: 6/8 of bf16.
  - Weights are pre-scaled by 256 (fp8e4 subnormal cutoff is 2^-6; raw
    weights have sigma 0.02), the Gaussian table by 4/256, and the host
    divides the output by 1024. All scales are powers of two (exact).

Phase structure per iteration (PE program order):
  warmup: discarded matmuls during the first DMA latency window;
  wave A: v[:, 0:512]   fp8 DoubleRow, banks 0-4 (bank=lt), kp-outer with
          passes HH,LH interleaved per kp and HL trailing (DMA streaming);
  wave B: v[:, 512:1024] banks [5,6,7,0,1], lt-outer, 12 matmuls/group;
  ph2:    attendedT per head, bf16 banded Toeplitz windows, banks [2,3,4,5];
          PSUM->SBUF copy to bf16 (DVE evens / Act odds) releases banks;
          fp8 hi/lo split runs in SBUF on DVE (evens) and Pool (odds);
  ph3:    outT = W2' @ attendedT, all 8 banks (bank=m); hi-consuming passes
          (HH then HL) kp-outer first, lo-consuming pass (LH) m-outer last
          so the fp8 splits hide behind ~7us of matmuls.
"""

import math
from contextlib import ExitStack

import numpy as np
import ml_dtypes

import concourse.bass as bass
from concourse import mybir
from concourse.bass_utils import run_bass_kernel_spmd

# ---- NEFF disk cache (keyed by BIR hash) to avoid recompiling identical
# graphs in fresh processes ----
import hashlib
import os
import shutil

_NEFF_CACHE_DIR = os.environ.get("NEFF_CACHE_DIR", "/root/neff_cache")


def _install_neff_cache():
    import concourse.bass_utils as _bu
    import concourse.bass2jax as _b2j
    if getattr(_bu, "_neff_cache_installed", False):
        return
    orig = _bu.compile_bir_kernel

    def cached(bir_json, tmpdir, neff_name="file.neff"):
        cpath = None
        try:
            os.makedirs(_NEFF_CACHE_DIR, exist_ok=True)
            key = hashlib.sha256(bir_json).hexdigest()[:32]
            cpath = os.path.join(_NEFF_CACHE_DIR, f"{key}.neff")
            dst = os.path.join(tmpdir, neff_name)
            if os.path.exists(cpath):
                shutil.copy(cpath, dst)
                return dst
        except OSError:
            cpath = None  # cache unusable; plain compile below
        path = orig(bir_json, tmpdir, neff_name)
        if cpath is not None:
            try:
                shutil.copy(path, cpath)
            except OSError:
                pass
        return path

    _bu.compile_bir_kernel = cached
    _b2j.compile_bir_kernel = cached
    _bu._neff_cache_installed = True


_install_neff_cache()

# ---------------- problem constants (hardcoded per spec) ----------------
B, L, D = 2, 2048, 1024
H, PD = 8, 128
ATTN_STD = np.array([1.0, 2.0, 4.0, 8.0, 1.0, 2.0, 4.0, 8.0], dtype=np.float64)
ATTN_OFFSET = np.array([-1.0, -2.0, -4.0, -8.0, -1.0, -2.0, -4.0, -8.0], dtype=np.float64)

N_CORES = 8
CHUNK = 512            # output rows per core
HALO_L, HALO_R = 56, 40
LPAD = 640             # 56 + 512 + 40 = 608, padded to 5*128
LREAL = 608            # rows actually shipped; SBUF cols 608:640 memset once
LT = 5                 # l-tiles of v (640 / 128)
KT = 8                 # d tiles (1024 / 128)
KP = 4                 # DoubleRow k-pairs (1024 / 256)
NQ = CHUNK             # query columns per core

BF16 = mybir.dt.bfloat16
F8 = mybir.dt.float8e4
F32 = mybir.dt.float32
DR = mybir.MatmulPerfMode.DoubleRow
NPF8 = ml_dtypes.float8_e4m3
NPBF = ml_dtypes.bfloat16

# power-of-two scales: weights *256 (clear fp8 subnormals), Gaussian table
# *4/256 (S1 cancel + attended into fp8 sweet spot), host output /(4*256)
S_W = 256.0
S_A = 4.0
OUT_DESCALE = 1.0 / (S_A * S_W)

TP0, TPW = 408, 256        # banded Toeplitz window (512B rows: no DMA penalty)

WB = [5, 6, 7, 0, 1]       # wave B bank per lt
PH2_BANKS = [2, 3, 4, 5, 0, 1, 2, 3]   # ph2 bank per head (6 banks used)

# p-state warmup: discarded matmul sizes (rows), burned during the first
# DMA latency window; tuned so the PE reaches the first wave-A wait just
# after data-readiness
WARMUP = (256, 256, 256, 184)


def gauss_toeplitz_table() -> np.ndarray:
    """tp[h, r, m] = g_h(r - (m - 512) - 56) * S_A/S_W, shape [H,128,1024] bf16.

    For v-tile t (rows k' = 128t + r of padded-local v) the attention rhs is
    tp[h][:, 512-128t : 1024-128t] so that rhs[r, q'] = g_h(128t + r - q' - 56),
    which is probs[h, q, k].T in padded-local coordinates.
    """
    r = np.arange(128, dtype=np.float64)[:, None]
    m = np.arange(1024, dtype=np.float64)[None, :]
    delta = r - (m - 512.0) - 56.0  # = k - q
    tables = []
    for h in range(H):
        std, off = ATTN_STD[h], ATTN_OFFSET[h]
        z = (delta - off) / std
        g = np.exp(-0.5 * z * z) / (std * math.sqrt(2.0 * math.pi))
        g[np.abs(z) > 6.0] = 0.0
        tables.append(g * (S_A / S_W))
    full = np.stack(tables).astype(NPBF)          # [H, 128, 1024]
    return np.ascontiguousarray(full[:, :, TP0:TP0 + TPW])


def attn_windows(h: int):
    """Static (t, j0, j1) list: nonzero q-column window of v-tile t for head h,
    8-aligned. Coverage of [0,512) is guaranteed (window width > 128)."""
    std, off = int(ATTN_STD[h]), int(ATTN_OFFSET[h])
    wlo = -56 - off - 6 * std
    whi = 71 - off + 6 * std
    res = []
    for t in range(LT):
        j0 = max(0, 128 * t + wlo)
        j1 = min(NQ, 128 * t + whi + 1)
        if j0 >= j1:
            continue
        j0 = (j0 // 8) * 8
        j1 = min(NQ, ((j1 + 7) // 8) * 8)
        res.append((t, j0, j1))
    return res


def build_graph(iters: int = 1, banded: bool = True) -> bass.Bass:
    """One SPMD core program. iters>1 repeats the whole kernel (including
    DMAs) with monotonically increasing semaphore thresholds, for timing."""
    nc = bass.Bass()

    xt = nc.declare_dram_parameter("xt", [2, D, LREAL], F8, isOutput=False)
    w1 = nc.declare_dram_parameter("w1", [2, D, D], F8, isOutput=False)
    w2 = nc.declare_dram_parameter("w2", [2, D, D], F8, isOutput=False)
    tp = nc.declare_dram_parameter("tp", [H, 128, TPW], BF16, isOutput=False)
    out = nc.declare_dram_parameter("out", [D, NQ], BF16, isOutput=True)

    xt_r = xt[:].rearrange("s (o p) f -> p s o f", p=128)   # [128, 2, 8, 608]
    w1_r = w1[:].rearrange("s (o p) f -> p s o f", p=128)   # [128, 2, 8, 1024]
    w2_r = w2[:].rearrange("s (o p) f -> p s o f", p=128)   # [128, 2, 8, 1024]
    tp_r = tp[:].rearrange("h p f -> p h f")                # [128, 8, 256]

    with ExitStack() as ctx:
        e = ctx.enter_context
        xt_sb = e(nc.sbuf_tensor("xt_sb", [128, 2, 2, KT, LPAD], F8))
        w1_sb = e(nc.sbuf_tensor("w1_sb", [128, 2, 2, KT, D], F8))
        w2_sb = e(nc.sbuf_tensor("w2_sb", [128, 2, KT, D], F8))
        tp_sb = e(nc.sbuf_tensor("tp_sb", [128, H, TPW], BF16))
        tp_src = tp_r[:, :, :]
        v_sb = e(nc.sbuf_tensor("v_sb", [128, LT, D], BF16))
        ab_sb = e(nc.sbuf_tensor("ab_sb", [128, H, NQ], BF16))   # attended bf16
        a8_sb = e(nc.sbuf_tensor("a8_sb", [128, 2, KT, NQ], F8))  # hi/lo fp8
        o_sb = e(nc.sbuf_tensor("o_sb", [128, KT, NQ], BF16))
        zdum = e(nc.sbuf_tensor("zdum", [128, 384], BF16))
        ps = [e(nc.psum_tensor(f"ps{i}", [128, 512], F32)) for i in range(8)]

        sem_names = (["zd", "xz", "mmA", "mm1", "mm2", "mm3", "cp37v", "tp_d",
                      "cpA", "cpB", "cp2v", "cp2s", "cp3v", "cp3s",
                      "phiV", "phiP", "ploV", "ploP", "w2h", "w2l"]
                     + [f"{n}b{p}" for n in ("xh01", "xh23", "xl01", "xl23",
                                             "wah01", "wah23", "wal01", "wal23",
                                             "wbh", "wbl") for p in (0, 1)]
                     + [f"dmo{m}" for m in range(KT)])
        sems = {n: e(nc.semaphore(n)) for n in sem_names}

        def cp2_sem(h):
            return sems["cp2v" if h % 2 == 0 else "cp2s"]

        def cp2_count(h, it):
            return it * 4 + h // 2 + 1

        def cp3_sem(m):
            return sems["cp3v" if m % 2 == 0 else "cp3s"]

        def cp3_count(m, it):
            return it * 4 + m // 2 + 1

        with nc.Block() as block:

            @block.sync
            def _(sync: bass.BassEngine):
                for it in range(iters):
                    buf = it % 2
                    if it > 1:
                        # xt/w1 buffer reuse: wave B HL (last reader) of it-2
                        sync.wait_ge(sems["mm1"], (it - 1) * LT)

                    def dma(dst, src, sem):
                        sync.dma_start(out=dst, in_=src).then_inc(sems[sem], 16)

                    # coarse chunks, issued in wave A consumption order
                    # (the shared HWDGE generator costs 625ns per issue)
                    dma(xt_sb[:, buf, 0, 0:4, 0:LREAL], xt_r[:, 0, 0:4, :],
                        f"xh01b{buf}")
                    dma(w1_sb[:, buf, 0, 0:4, 0:512], w1_r[:, 0, 0:4, 0:512],
                        f"wah01b{buf}")
                    dma(xt_sb[:, buf, 0, 4:8, 0:LREAL], xt_r[:, 0, 4:8, :],
                        f"xh23b{buf}")
                    dma(w1_sb[:, buf, 0, 4:8, 0:512], w1_r[:, 0, 4:8, 0:512],
                        f"wah23b{buf}")
                    dma(xt_sb[:, buf, 1, 0:4, 0:LREAL], xt_r[:, 1, 0:4, :],
                        f"xl01b{buf}")
                    dma(xt_sb[:, buf, 1, 4:8, 0:LREAL], xt_r[:, 1, 4:8, :],
                        f"xl23b{buf}")
                    dma(w1_sb[:, buf, 1, 0:4, 0:512], w1_r[:, 1, 0:4, 0:512],
                        f"wal01b{buf}")
                    dma(w1_sb[:, buf, 1, 4:8, 0:512], w1_r[:, 1, 4:8, 0:512],
                        f"wal23b{buf}")
                    dma(w1_sb[:, buf, 0, :, 512:1024], w1_r[:, 0, :, 512:1024],
                        f"wbhb{buf}")
                    dma(w1_sb[:, buf, 1, :, 512:1024], w1_r[:, 1, :, 512:1024],
                        f"wblb{buf}")
                    if it == 0:
                        sync.dma_start(out=tp_sb[:], in_=tp_src).then_inc(
                            sems["tp_d"], 16)
                    if it > 0:
                        sync.wait_ge(sems["mm3"], it * KT)
                    dma(w2_sb[:, 0, :, :], w2_r[:, 0, :, :], "w2h")
                    dma(w2_sb[:, 1, :, :], w2_r[:, 1, :, :], "w2l")

            @block.tensor
            def _(tensor: bass.BassEngine):
                # HAM/p-state warmup: discarded matmuls into bank 0 while the
                # first input DMAs are in flight
                tensor.wait_ge(sems["zd"], 1)
                for wn in WARMUP:
                    tensor.matmul(ps[0][:, 0:wn], zdum[:, 0:128],
                                  zdum[:, 128:128 + wn], start=True, stop=True)

                def dr_mm(bank, lhsT, rhs, start, stop):
                    return tensor.matmul(bank[:, :], lhsT, rhs,
                                         start=start, stop=stop, perf_mode=DR)

                for it in range(iters):
                    buf = it % 2
                    nth = (it // 2 + 1) * 16  # per-parity DMA count

                    # ---- wave A: v[:, 0:512], banks 0-4 (bank = lt) ----
                    # kp-outer; passes HH (x_hi*w_hi) and LH (x_lo*w_hi)
                    # interleaved per kp to match DMA arrival order; HL
                    # (x_hi*w_lo) trails.
                    for t in range(KP):
                        if t == 0:
                            if it == 0:
                                tensor.wait_ge(sems["xz"], 1)
                            tensor.wait_ge(sems[f"xh01b{buf}"], nth)
                            tensor.wait_ge(sems[f"wah01b{buf}"], nth)
                        elif t == 2:
                            tensor.wait_ge(sems[f"xh23b{buf}"], nth)
                            tensor.wait_ge(sems[f"wah23b{buf}"], nth)
                        for lt in range(LT):
                            if t == 0 and it > 0:
                                # bank lt <- ph3 m=lt copy of prev iter
                                tensor.wait_ge(cp3_sem(lt), cp3_count(lt, it - 1))
                            dr_mm(ps[lt],
                                  xt_sb[:, buf, 0, 2 * t:2 * t + 2,
                                        128 * lt:128 * lt + 128],
                                  w1_sb[:, buf, 0, 2 * t:2 * t + 2, 0:512],
                                  start=(t == 0), stop=False)
                    for t in range(KP):
                        if t == 0:
                            tensor.wait_ge(sems[f"xl01b{buf}"], nth)
                        elif t == 2:
                            tensor.wait_ge(sems[f"xl23b{buf}"], nth)
                        for lt in range(LT):
                            dr_mm(ps[lt],
                                  xt_sb[:, buf, 1, 2 * t:2 * t + 2,
                                        128 * lt:128 * lt + 128],
                                  w1_sb[:, buf, 0, 2 * t:2 * t + 2, 0:512],
                                  start=False, stop=False)
                    for t in range(KP):
                        if t == 0:
                            tensor.wait_ge(sems[f"wal01b{buf}"], nth)
                        elif t == 2:
                            tensor.wait_ge(sems[f"wal23b{buf}"], nth)
                        for lt in range(LT):
                            mm = dr_mm(ps[lt],
                                       xt_sb[:, buf, 0, 2 * t:2 * t + 2,
                                             128 * lt:128 * lt + 128],
                                       w1_sb[:, buf, 1, 2 * t:2 * t + 2, 0:512],
                                       start=False, stop=(t == KP - 1))
                            if t == KP - 1:
                                mm.then_inc(sems["mmA"])

                    # ---- wave B: v[:, 512:1024], banks [5,6,7,0,1], lt-outer --
                    for lt in range(LT):
                        bank = ps[WB[lt]]
                        if it > 0 and lt == 0:
                            tensor.wait_ge(cp3_sem(5), cp3_count(5, it - 1))
                        elif it > 0 and lt == 1:
                            tensor.wait_ge(cp3_sem(6), cp3_count(6, it - 1))
                        elif it > 0 and lt == 2:
                            tensor.wait_ge(cp3_sem(7), cp3_count(7, it - 1))
                            tensor.wait_ge(sems["cp37v"], it)
                        elif lt == 3:
                            tensor.wait_ge(sems["cpA"], it * LT + 1)
                        elif lt == 4:
                            tensor.wait_ge(sems["cpA"], it * LT + 2)
                        for pas in range(3):  # HH, LH, HL
                            sx = 1 if pas == 1 else 0
                            sw = 1 if pas == 2 else 0
                            for t in range(KP):
                                if lt == 0 and pas == 0 and t == 0:
                                    tensor.wait_ge(sems[f"wbhb{buf}"], nth)
                                if lt == 0 and pas == 2 and t == 0:
                                    tensor.wait_ge(sems[f"wblb{buf}"], nth)
                                mm = dr_mm(
                                    bank,
                                    xt_sb[:, buf, sx, 2 * t:2 * t + 2,
                                          128 * lt:128 * lt + 128],
                                    w1_sb[:, buf, sw, 2 * t:2 * t + 2, 512:1024],
                                    start=(pas == 0 and t == 0),
                                    stop=(pas == 2 and t == KP - 1))
                                if pas == 2 and t == KP - 1:
                                    mm.then_inc(sems["mm1"])

                    # ---- phase 2: attendedT per head (bf16 banded) ----
                    if it == 0:
                        tensor.wait_ge(sems["tp_d"], 16)
                    for h in range(H):
                        bank = ps[PH2_BANKS[h]]
                        # bank WAR: banks 2,3,4 <- wave A lt=2,3,4 copies;
                        # bank 5 <- wave B lt=0 copy; banks 0,1 <- wave B
                        # lt=3,4 copies; h>=6 <- ph2 head h-6 copy
                        if h == 0:
                            tensor.wait_ge(sems["cpA"], it * LT + 3)
                        elif h == 1:
                            tensor.wait_ge(sems["cpA"], it * LT + 4)
                        elif h == 2:
                            tensor.wait_ge(sems["cpA"], it * LT + 5)
                        elif h == 3:
                            tensor.wait_ge(sems["cpB"], it * LT + 1)
                        elif h == 4:
                            tensor.wait_ge(sems["cpB"], it * LT + 4)
                        elif h == 5:
                            tensor.wait_ge(sems["cpB"], it * LT + 5)
                        else:
                            tensor.wait_ge(cp2_sem(h - 6), cp2_count(h - 6, it))
                        windows = attn_windows(h)
                        for wi, (t, j0, j1) in enumerate(windows):
                            if h // 4 == 0:
                                tensor.wait_ge(sems["cpA"], it * LT + t + 1)
                            else:
                                tensor.wait_ge(sems["cpB"], it * LT + t + 1)
                            c0 = 512 - 128 * t + j0 - TP0
                            c1 = 512 - 128 * t + j1 - TP0
                            mm = tensor.matmul(
                                bank[:, j0:j1],
                                v_sb[:, t, 128 * h:128 * h + 128],
                                tp_sb[:, h, c0:c1],
                                start=(wi == 0), stop=(wi == len(windows) - 1),
                            )
                            if wi == len(windows) - 1:
                                mm.then_inc(sems["mm2"])

                    # ---- phase 3: outT = W2' @ attendedT, banks 0-7 ----
                    # stage 1 (hi-consuming) is fully kp-outer so the at8-hi
                    # for pair t is needed only ~1.7us*t after ph3 starts,
                    # matching the pace the splits come off the copy engines.
                    # Per kp: HH/HL over m-set A (banks from wave B copies),
                    # then over m-set B (banks from ph2 copies, later).
                    # stage 2 (LH, lo-consuming) is m-outer and closes groups.
                    MSET_A = (6, 7, 4, 5)
                    MSET_B = (0, 1, 2, 3)
                    for t in range(KP):
                        tensor.wait_ge(sems["phiV"], it * 4 + t + 1)
                        tensor.wait_ge(sems["phiP"], it * 4 + t + 1)
                        if t == 0:
                            tensor.wait_ge(sems["w2h"], (it + 1) * 16)
                        for si, mset in enumerate((MSET_A, MSET_B)):
                            for pas in range(2):  # HH, HL
                                if t == 0 and si == 0 and pas == 1:
                                    tensor.wait_ge(sems["w2l"], (it + 1) * 16)
                                for m in mset:
                                    if t == 0 and pas == 0:
                                        # bank WAR: last ph2 user of bank m
                                        if m == 6:
                                            tensor.wait_ge(sems["cpB"],
                                                           it * LT + 2)
                                        elif m == 7:
                                            tensor.wait_ge(sems["cpB"],
                                                           it * LT + 3)
                                        else:
                                            hh = {0: 4, 1: 5, 2: 6, 3: 7,
                                                  4: 2, 5: 3}[m]
                                            tensor.wait_ge(cp2_sem(hh),
                                                           cp2_count(hh, it))
                                    dr_mm(ps[m],
                                          w2_sb[:, pas, 2 * t:2 * t + 2,
                                                128 * m:128 * m + 128],
                                          a8_sb[:, 0, 2 * t:2 * t + 2, :],
                                          start=(t == 0 and pas == 0),
                                          stop=False)
                    for m in range(KT):  # LH pass, closes groups
                        for t in range(KP):
                            if m == 0:
                                tensor.wait_ge(sems["ploV"], it * 4 + t + 1)
                                tensor.wait_ge(sems["ploP"], it * 4 + t + 1)
                            mm = dr_mm(ps[m],
                                       w2_sb[:, 0, 2 * t:2 * t + 2,
                                             128 * m:128 * m + 128],
                                       a8_sb[:, 1, 2 * t:2 * t + 2, :],
                                       start=False, stop=(t == KP - 1))
                            if t == KP - 1:
                                mm.then_inc(sems["mm3"])

            @block.vector
            def _(vector: bass.BassEngine):
                for it in range(iters):
                    # wave A copies: v[:, lt, 0:512] from banks 0-4
                    for lt in range(LT):
                        vector.wait_ge(sems["mmA"], it * LT + lt + 1)
                        vector.tensor_copy(
                            out=v_sb[:, lt, 0:512], in_=ps[lt][:, :],
                        ).then_inc(sems["cpA"])
                    # ph2 even heads: bf16 copies (free banks) + fp8 hi
                    # splits, ordered so cp2v(h4)/cp2v(h6) (ph3 m-set B bank
                    # WARs) are not queued behind hi splits
                    def cp2(h):
                        vector.wait_ge(sems["mm2"], it * H + h + 1)
                        vector.tensor_copy(
                            out=ab_sb[:, h, :], in_=ps[PH2_BANKS[h]][:, :],
                        ).then_inc(sems["cp2v"])

                    def hi(h):
                        vector.tensor_copy(
                            out=a8_sb[:, 0, h, :], in_=ab_sb[:, h, :],
                        ).then_inc(sems["phiV"])

                    cp2(0); hi(0); cp2(2); cp2(4); hi(2); cp2(6); hi(4); hi(6)
                    for h in (0, 2, 4, 6):
                        vector.tensor_sub(
                            a8_sb[:, 1, h, :], ab_sb[:, h, :], a8_sb[:, 0, h, :],
                        ).then_inc(sems["ploV"])
                    for m in (0, 2, 4, 6):
                        vector.wait_ge(sems["mm3"], it * KT + m + 1)
                        if it > 0:
                            vector.wait_ge(sems[f"dmo{m}"], it * 16)
                        vector.tensor_copy(
                            out=o_sb[:, m, :], in_=ps[m][:, :],
                        ).then_inc(sems["cp3v"])
                    # first half of m7's copy, in parallel with Act's half
                    vector.wait_ge(sems["mm3"], it * KT + 8)
                    if it > 0:
                        vector.wait_ge(sems["dmo7"], it * 16)
                    vector.tensor_copy(
                        out=o_sb[:, 7, 0:256], in_=ps[7][:, 0:256],
                    ).then_inc(sems["cp37v"])

            @block.gpsimd
            def _(gpsimd: bass.BassEngine):
                # zero the PE-warmup tile before anything else
                gpsimd.memset(zdum[:], 0).then_inc(sems["zd"])
                # zero the untransferred xt pad rows (cols 608:640) once
                gpsimd.memset(xt_sb[:, :, :, :, LREAL:LPAD], 0).then_inc(
                    sems["xz"])
                for it in range(iters):
                    # ph2 odd heads: fp8 hi then lo splits (SBUF only)
                    for h in (1, 3, 5, 7):
                        gpsimd.wait_ge(sems["cp2s"], it * 4 + h // 2 + 1)
                        gpsimd.tensor_copy(
                            out=a8_sb[:, 0, h, :], in_=ab_sb[:, h, :],
                        ).then_inc(sems["phiP"])
                    for h in (1, 3, 5, 7):
                        gpsimd.tensor_sub(
                            a8_sb[:, 1, h, :], ab_sb[:, h, :], a8_sb[:, 0, h, :],
                        ).then_inc(sems["ploP"])
                    for m in range(7):
                        gpsimd.wait_ge(cp3_sem(m), cp3_count(m, it))
                        gpsimd.dma_start(
                            out=out[128 * m:128 * m + 128, :],
                            in_=o_sb[:, m, :],
                        ).then_inc(sems[f"dmo{m}"], 16)

                for m in range(KT):
                    gpsimd.wait_ge(sems[f"dmo{m}"], iters * 16)

            @block.scalar
            def _(scalar: bass.BassEngine):
                for it in range(iters):
                    # wave B copies: v[:, lt, 512:1024] from banks [5,6,7,0,1]
                    for lt in range(LT):
                        scalar.wait_ge(sems["mm1"], it * LT + lt + 1)
                        scalar.copy(v_sb[:, lt, 512:1024],
                                    ps[WB[lt]][:, :]).then_inc(sems["cpB"])
                    for h in (1, 3, 5, 7):
                        scalar.wait_ge(sems["mm2"], it * H + h + 1)
                        if it > 0:
                            # ab_sb[h] reuse: prev-iter lo split (Pool) done
                            scalar.wait_ge(sems["ploP"], (it - 1) * 4 + h // 2 + 1)
                        scalar.copy(ab_sb[:, h, :],
                                    ps[PH2_BANKS[h]][:, :]).then_inc(sems["cp2s"])
                    for m in (1, 3, 5):
                        scalar.wait_ge(sems["mm3"], it * KT + m + 1)
                        if it > 0:
                            scalar.wait_ge(sems[f"dmo{m}"], it * 16)
                        scalar.copy(o_sb[:, m, :],
                                    ps[m][:, :]).then_inc(sems["cp3s"])
                    # m7: halves copied on Act + DVE in parallel, then one
                    # store issued here (shorter tail than the Pool hop)
                    scalar.wait_ge(sems["mm3"], it * KT + 8)
                    if it > 0:
                        scalar.wait_ge(sems["dmo7"], it * 16)
                    scalar.copy(o_sb[:, 7, 256:512],
                                ps[7][:, 256:512]).then_inc(sems["cp3s"])
                    scalar.wait_ge(sems["cp37v"], it + 1)
                    scalar.dma_start(
                        out=out[896:1024, :], in_=o_sb[:, 7, :],
                    ).then_inc(sems["dmo7"], 16)

    return nc


# ---------------- host side ----------------

_GRAPH_CACHE: dict = {}


def get_graph(iters: int = 1, banded: bool = True) -> bass.Bass:
    key = (iters, banded)
    if key not in _GRAPH_CACHE:
        _GRAPH_CACHE[key] = build_graph(iters, banded)
    return _GRAPH_CACHE[key]


class Runner:
    """Compile-once executor for one Bass graph across the 8 cores.

    Mirrors bass2jax.run_bass_via_pjrt but keeps the jitted callable so
    repeated invocations don't re-trace/re-compile.
    """

    def __init__(self, nc: bass.Bass, n_cores: int = N_CORES):
        import jax
        from jax.sharding import Mesh, PartitionSpec
        from jax.experimental.shard_map import shard_map
        from concourse import bass2jax, mybir as _mb

        bass2jax.install_neuronx_cc_hook()
        self.n_cores = n_cores

        partition_name = (nc.partition_id_tensor.name
                          if nc.partition_id_tensor else None)
        in_names, out_names, out_avals, zero_shapes = [], [], [], []
        for alloc in nc.m.functions[0].allocations:
            if not isinstance(alloc, _mb.MemoryLocationSet):
                continue
            name = alloc.memorylocations[0].name
            if alloc.kind == "ExternalInput":
                if name != partition_name:
                    in_names.append(name)
            elif alloc.kind == "ExternalOutput":
                out_names.append(name)
                shape = tuple(alloc.tensor_shape)
                dtype = _mb.dt.np(alloc.dtype)
                out_avals.append(jax.core.ShapedArray(shape, dtype))
                zero_shapes.append((shape, dtype))
        self.in_names = list(in_names)
        self.out_names = out_names
        self.out_avals = out_avals
        self.zero_shapes = zero_shapes
        n_params = len(in_names)
        all_names = in_names + out_names
        if partition_name is not None:
            all_names = all_names + [partition_name]

        def _body(*args):
            operands = list(args)
            if partition_name is not None:
                operands.append(bass2jax.partition_id_tensor())
            outs = bass2jax._bass_exec_p.bind(
                *operands,
                out_avals=tuple(out_avals),
                in_names=tuple(all_names),
                out_names=tuple(out_names),
                lowering_input_output_aliases=(),
                sim_require_finite=True,
                sim_require_nnan=True,
                nc=nc,
            )
            return tuple(outs)

        devices = jax.devices()[:n_cores]
        mesh = Mesh(np.asarray(devices), ("core",))
        self._mesh = mesh
        n_outs = len(out_names)
        self._fn = jax.jit(
            shard_map(_body, mesh=mesh,
                      in_specs=(PartitionSpec("core"),) * (n_params + n_outs),
                      out_specs=(PartitionSpec("core"),) * n_outs,
                      check_rep=False),
            donate_argnums=tuple(range(n_params, n_params + n_outs)),
            keep_unused=True,
        )

    def stage(self, in_maps):
        """device_put the concatenated inputs once; returns device arrays."""
        import jax
        concat_in = [
            np.concatenate([np.asarray(m[name]) for m in in_maps], axis=0)
            for name in self.in_names
        ]
        return [jax.device_put(a) for a in concat_in]

    def make_zeros(self):
        if not hasattr(self, "_zeros_fn"):
            import jax
            import jax.numpy as jnp
            from jax.sharding import NamedSharding, PartitionSpec
            shardings = tuple(
                NamedSharding(self._mesh, PartitionSpec("core"))
                for _ in self.zero_shapes)
            shapes = [((self.n_cores * s[0], *s[1:]), d)
                      for s, d in self.zero_shapes]

            def _mk():
                return tuple(jnp.zeros(sh, dt) for sh, dt in shapes)

            self._zeros_fn = jax.jit(_mk, out_shardings=shardings)
        return list(self._zeros_fn())

    def run_staged(self, dev_in, dev_zeros):
        return self._fn(*dev_in, *dev_zeros)

    def __call__(self, in_maps):
        out_arrs = self._fn(*self.stage(in_maps), *self.make_zeros())
        return [
            {name: np.asarray(out_arrs[i]).reshape(
                self.n_cores, *self.out_avals[i].shape)[c]
             for i, name in enumerate(self.out_names)}
            for c in range(self.n_cores)
        ]


_RUNNER_CACHE: dict = {}


def get_runner(iters: int = 1) -> "Runner":
    if iters not in _RUNNER_CACHE:
        _RUNNER_CACHE[iters] = Runner(get_graph(iters))
    return _RUNNER_CACHE[iters]


def _split8(a: np.ndarray) -> np.ndarray:
    """[2, ...] stack of (hi, lo) fp8e4 halves of a float32 array."""
    hi = a.astype(np.float32).astype(NPF8)
    lo = (a.astype(np.float32) - hi.astype(np.float32)).astype(NPF8)
    return np.stack([hi, lo])


def make_in_maps(values: np.ndarray, input_weights: np.ndarray,
                 output_weight: np.ndarray) -> list:
    w1s = _split8(np.ascontiguousarray(input_weights.T) * S_W)
    w2s = _split8(np.ascontiguousarray(output_weight.T) * S_W)
    tpt = gauss_toeplitz_table()
    in_maps = []
    for core in range(N_CORES):
        b, c = divmod(core, 4)
        lo, hi = c * CHUNK - HALO_L, c * CHUNK + CHUNK + HALO_R
        src_lo, src_hi = max(lo, 0), min(hi, L)
        xt_pad = np.zeros((D, LREAL), dtype=np.float32)
        xt_pad[:, src_lo - lo:src_hi - lo] = values[b, src_lo:src_hi, :].T
        in_maps.append({"xt": _split8(xt_pad), "w1": w1s, "w2": w2s, "tp": tpt})
    return in_maps


def assemble(results: list) -> np.ndarray:
    out = np.empty((B, L, D), dtype=np.float32)
    for core in range(N_CORES):
        b, c = divmod(core, 4)
        out[b, c * CHUNK:(c + 1) * CHUNK, :] = \
            results[core]["out"].T.astype(np.float32) * OUT_DESCALE
    return out


def kernel(values: np.ndarray, input_weights: np.ndarray,
           output_weight: np.ndarray) -> np.ndarray:
    in_maps = make_in_maps(values, input_weights, output_weight)
    try:
        return assemble(get_runner(1)(in_maps))
    except Exception:
        # fallback: canonical SPMD path (re-traces per call but always works)
        res = run_bass_kernel_spmd(get_graph(1), in_maps,
                                   core_ids=list(range(N_CORES)))
        return assemble(res.results)


# revision 44
# speedup vs baseline: 1.1968x; 1.0201x over previous
"""Trainium2 Bass kernel for nn_Attention (Gaussian banded attention).

Math (reference):
    v = values @ input_weights.T                      # [B,L,D]
    probs[h,q,k] = N(k - q - off_h; std_h)            # Gaussian, depends on k-q only
    attended[b,h,q,:] = sum_k probs[h,q,k] v[b,k,h*pd:(h+1)*pd]
    out = attended_merged @ output_weight.T           # [B,L,D]

Structural facts exploited:
  - probs is banded Toeplitz per head (6-sigma truncation) -> attention is a
    narrow depthwise conv along L, done as windowed matmuls vs a 128x1024
    Toeplitz table. Batch x L sharding is embarrassingly parallel with a
    56/40-row input halo (8 cores = 2 batches x 4 chunks of 512 rows).
  - The two dense 1024x1024 projections dominate PE time. They run as
    fp8e4(DoubleRow) matmuls: each instruction contracts 2x128 rows at
    0.5 cycles/row = 4x bf16 throughput. Full bf16-level precision is kept
    by splitting each operand x = hi + lo (both fp8) and accumulating
    three of the four cross terms in PSUM (hi*hi + lo*hi + hi*lo); the
    dropped lo*lo term is ~0.1% relative. Net projection cost: 6/8 of bf16.
  - Weights are pre-scaled by 256 (fp8e4 subnormal cutoff is 2^-6; raw
    weights have sigma 0.02), the Gaussian table by 4/256, and the host
    divides the output by 1024. All scales are powers of two (exact).

Phase structure per iteration (PE program order):
  warmup: discarded matmuls during the first DMA latency window;
  wave A: v[:, 0:512]   fp8 DoubleRow, banks 0-4 (bank=lt), kp-outer with
          passes HH,LH interleaved per kp and HL trailing (DMA streaming);
  wave B: v[:, 512:1024] banks [5,6,7,0,1], lt-outer, 12 matmuls/group;
  ph2:    attendedT per head, bf16 banded Toeplitz windows, banks [2,3,4,5];
          PSUM->SBUF copy to bf16 (DVE evens / Act odds) releases banks;
          fp8 hi/lo split runs in SBUF on DVE (evens) and Pool (odds);
  ph3:    outT = W2' @ attendedT, all 8 banks (bank=m); hi-consuming passes
          (HH then HL) kp-outer first, lo-consuming pass (LH) m-outer last
          so the fp8 splits hide behind ~7us of matmuls.
"""

import math
from contextlib import ExitStack

import numpy as np
import ml_dtypes

import concourse.bass as bass
from concourse import mybir
from concourse.bass_utils import run_bass_kernel_spmd

# ---- NEFF disk cache (keyed by BIR hash) to avoid recompiling identical
# graphs in fresh processes ----
import hashlib
import os
import shutil

_NEFF_CACHE_DIR = os.environ.get("NEFF_CACHE_DIR", "/root/neff_cache")


def _install_neff_cache():
    import concourse.bass_utils as _bu
    import concourse.bass2jax as _b2j
    if getattr(_bu, "_neff_cache_installed", False):
        return
    orig = _bu.compile_bir_kernel

    def cached(bir_json, tmpdir, neff_name="file.neff"):
        cpath = None
        try:
            os.makedirs(_NEFF_CACHE_DIR, exist_ok=True)
            key = hashlib.sha256(bir_json).hexdigest()[:32]
            cpath = os.path.join(_NEFF_CACHE_DIR, f"{key}.neff")
            dst = os.path.join(tmpdir, neff_name)
            if os.path.exists(cpath):
                shutil.copy(cpath, dst)
                return dst
        except OSError:
            cpath = None  # cache unusable; plain compile below
        path = orig(bir_json, tmpdir, neff_name)
        if cpath is not None:
            try:
                shutil.copy(path, cpath)
            except OSError:
                pass
        return path

    _bu.compile_bir_kernel = cached
    _b2j.compile_bir_kernel = cached
    _bu._neff_cache_installed = True


_install_neff_cache()

# ---------------- problem constants (hardcoded per spec) ----------------
B, L, D = 2, 2048, 1024
H, PD = 8, 128
ATTN_STD = np.array([1.0, 2.0, 4.0, 8.0, 1.0, 2.0, 4.0, 8.0], dtype=np.float64)
ATTN_OFFSET = np.array([-1.0, -2.0, -4.0, -8.0, -1.0, -2.0, -4.0, -8.0], dtype=np.float64)

N_CORES = 8
CHUNK = 512            # output rows per core
HALO_L, HALO_R = 56, 40
LPAD = 640             # 56 + 512 + 40 = 608, padded to 5*128
LREAL = 608            # rows actually shipped; SBUF cols 608:640 memset once
LT = 5                 # l-tiles of v (640 / 128)
KT = 8                 # d tiles (1024 / 128)
KP = 4                 # DoubleRow k-pairs (1024 / 256)
NQ = CHUNK             # query columns per core

BF16 = mybir.dt.bfloat16
F8 = mybir.dt.float8e4
F32 = mybir.dt.float32
DR = mybir.MatmulPerfMode.DoubleRow
NPF8 = ml_dtypes.float8_e4m3
NPBF = ml_dtypes.bfloat16

# power-of-two scales: weights *256 (clear fp8 subnormals), Gaussian table
# *4/256 (S1 cancel + attended into fp8 sweet spot), host output /(4*256)
S_W = 256.0
S_A = 4.0
OUT_DESCALE = 1.0 / (S_A * S_W)

TP0, TPW = 408, 256        # banded Toeplitz window (512B rows: no DMA penalty)

WB = [5, 6, 7, 0, 1]       # wave B bank per lt
PH2_BANKS = [2, 3, 4, 5, 0, 1, 2, 3]   # ph2 bank per head (6 banks used)

# p-state warmup: discarded matmul sizes (rows), burned during the first
# DMA latency window; tuned so the PE reaches the first wave-A wait just
# after data-readiness
WARMUP = (256, 256, 256, 184)


def gauss_toeplitz_table() -> np.ndarray:
    """tp[h, r, m] = g_h(r - (m - 512) - 56) * S_A/S_W, shape [H,128,1024] bf16.

    For v-tile t (rows k' = 128t + r of padded-local v) the attention rhs is
    tp[h][:, 512-128t : 1024-128t] so that rhs[r, q'] = g_h(128t + r - q' - 56),
    which is probs[h, q, k].T in padded-local coordinates.
    """
    r = np.arange(128, dtype=np.float64)[:, None]
    m = np.arange(1024, dtype=np.float64)[None, :]
    delta = r - (m - 512.0) - 56.0  # = k - q
    tables = []
    for h in range(H):
        std, off = ATTN_STD[h], ATTN_OFFSET[h]
        z = (delta - off) / std
        g = np.exp(-0.5 * z * z) / (std * math.sqrt(2.0 * math.pi))
        g[np.abs(z) > 6.0] = 0.0
        tables.append(g * (S_A / S_W))
    full = np.stack(tables).astype(NPBF)          # [H, 128, 1024]
    return np.ascontiguousarray(full[:, :, TP0:TP0 + TPW])


def attn_windows(h: int):
    """Static (t, j0, j1) list: nonzero q-column window of v-tile t for head h,
    8-aligned. Coverage of [0,512) is guaranteed (window width > 128)."""
    std, off = int(ATTN_STD[h]), int(ATTN_OFFSET[h])
    wlo = -56 - off - 6 * std
    whi = 71 - off + 6 * std
    res = []
    for t in range(LT):
        j0 = max(0, 128 * t + wlo)
        j1 = min(NQ, 128 * t + whi + 1)
        if j0 >= j1:
            continue
        j0 = (j0 // 8) * 8
        j1 = min(NQ, ((j1 + 7) // 8) * 8)
        res.append((t, j0, j1))
    return res


def build_graph(iters: int = 1, banded: bool = True) -> bass.Bass:
    """One SPMD core program. iters>1 repeats the whole kernel (including
    DMAs) with monotonically increasing semaphore thresholds, for timing."""
    nc = bass.Bass()

    xt = nc.declare_dram_parameter("xt", [2, D, LREAL], F8, isOutput=False)
    w1 = nc.declare_dram_parameter("w1", [2, D, D], F8, isOutput=False)
    w2 = nc.declare_dram_parameter("w2", [2, D, D], F8, isOutput=False)
    tp = nc.declare_dram_parameter("tp", [H, 128, TPW], BF16, isOutput=False)
    out = nc.declare_dram_parameter("out", [D, NQ], BF16, isOutput=True)

    xt_r = xt[:].rearrange("s (o p) f -> p s o f", p=128)   # [128, 2, 8, 608]
    w1_r = w1[:].rearrange("s (o p) f -> p s o f", p=128)   # [128, 2, 8, 1024]
    w2_r = w2[:].rearrange("s (o p) f -> p s o f", p=128)   # [128, 2, 8, 1024]
    tp_r = tp[:].rearrange("h p f -> p h f")                # [128, 8, 256]

    with ExitStack() as ctx:
        e = ctx.enter_context
        xt_sb = e(nc.sbuf_tensor("xt_sb", [128, 2, 2, KT, LPAD], F8))
        w1_sb = e(nc.sbuf_tensor("w1_sb", [128, 2, 2, KT, D], F8))
        w2_sb = e(nc.sbuf_tensor("w2_sb", [128, 2, KT, D], F8))
        tp_sb = e(nc.sbuf_tensor("tp_sb", [128, H, TPW], BF16))
        tp_src = tp_r[:, :, :]
        v_sb = e(nc.sbuf_tensor("v_sb", [128, LT, D], BF16))
        ab_sb = e(nc.sbuf_tensor("ab_sb", [128, H, NQ], BF16))   # attended bf16
        a8_sb = e(nc.sbuf_tensor("a8_sb", [128, 2, KT, NQ], F8))  # hi/lo fp8
        o_sb = e(nc.sbuf_tensor("o_sb", [128, KT, NQ], BF16))
        zdum = e(nc.sbuf_tensor("zdum", [128, 384], BF16))
        ps = [e(nc.psum_tensor(f"ps{i}", [128, 512], F32)) for i in range(8)]

        sem_names = (["zd", "xz", "mmA", "mm1", "mm2", "mm3", "tp_d",
                      "cpA", "cpB", "cp2v", "cp2s", "cp3v", "cp3s",
                      "phiV", "phiP", "ploV", "ploP", "w2h", "w2l"]
                     + [f"{n}b{p}" for n in ("xh01", "xh23", "xl01", "xl23",
                                             "wah01", "wah23", "wal01", "wal23",
                                             "wbh", "wbl") for p in (0, 1)]
                     + [f"dmo{m}" for m in range(KT)] + ["dmo6f"])
        sems = {n: e(nc.semaphore(n)) for n in sem_names}

        def cp2_sem(h):
            return sems["cp2v" if h % 2 == 0 else "cp2s"]

        def cp2_count(h, it):
            return it * 4 + h // 2 + 1

        def cp3_sem(m):
            return sems["cp3v" if m % 2 == 0 else "cp3s"]

        def cp3_count(m, it):
            return it * 4 + m // 2 + 1

        with nc.Block() as block:

            @block.sync
            def _(sync: bass.BassEngine):
                for it in range(iters):
                    buf = it % 2
                    if it > 1:
                        # xt/w1 buffer reuse: wave B HL (last reader) of it-2
                        sync.wait_ge(sems["mm1"], (it - 1) * LT)

                    def dma(dst, src, sem):
                        sync.dma_start(out=dst, in_=src).then_inc(sems[sem], 16)

                    # coarse chunks, issued in wave A consumption order
                    # (the shared HWDGE generator costs 625ns per issue)
                    dma(xt_sb[:, buf, 0, 0:4, 0:LREAL], xt_r[:, 0, 0:4, :],
                        f"xh01b{buf}")
                    dma(w1_sb[:, buf, 0, 0:4, 0:512], w1_r[:, 0, 0:4, 0:512],
                        f"wah01b{buf}")
                    dma(xt_sb[:, buf, 0, 4:8, 0:LREAL], xt_r[:, 0, 4:8, :],
                        f"xh23b{buf}")
                    dma(w1_sb[:, buf, 0, 4:8, 0:512], w1_r[:, 0, 4:8, 0:512],
                        f"wah23b{buf}")
                    dma(xt_sb[:, buf, 1, 0:4, 0:LREAL], xt_r[:, 1, 0:4, :],
                        f"xl01b{buf}")
                    dma(xt_sb[:, buf, 1, 4:8, 0:LREAL], xt_r[:, 1, 4:8, :],
                        f"xl23b{buf}")
                    dma(w1_sb[:, buf, 1, 0:4, 0:512], w1_r[:, 1, 0:4, 0:512],
                        f"wal01b{buf}")
                    dma(w1_sb[:, buf, 1, 4:8, 0:512], w1_r[:, 1, 4:8, 0:512],
                        f"wal23b{buf}")
                    dma(w1_sb[:, buf, 0, :, 512:1024], w1_r[:, 0, :, 512:1024],
                        f"wbhb{buf}")
                    dma(w1_sb[:, buf, 1, :, 512:1024], w1_r[:, 1, :, 512:1024],
                        f"wblb{buf}")
                    if it == 0:
                        sync.dma_start(out=tp_sb[:], in_=tp_src).then_inc(
                            sems["tp_d"], 16)
                    if it > 0:
                        sync.wait_ge(sems["mm3"], it * KT)
                    dma(w2_sb[:, 0, :, :], w2_r[:, 0, :, :], "w2h")
                    dma(w2_sb[:, 1, :, :], w2_r[:, 1, :, :], "w2l")
                    if it == iters - 1:
                        sync.wait_ge(sems["cp3v"], it * 4 + 4)
                        sync.dma_start(
                            out=out[768:896, :], in_=o_sb[:, 6, :],
                        ).then_inc(sems["dmo6f"], 16)

            @block.tensor
            def _(tensor: bass.BassEngine):
                # HAM/p-state warmup: discarded matmuls into bank 0 while the
                # first input DMAs are in flight
                tensor.wait_ge(sems["zd"], 1)
                for wn in WARMUP:
                    tensor.matmul(ps[0][:, 0:wn], zdum[:, 0:128],
                                  zdum[:, 128:128 + wn], start=True, stop=True)

                def dr_mm(bank, lhsT, rhs, start, stop):
                    return tensor.matmul(bank[:, :], lhsT, rhs,
                                         start=start, stop=stop, perf_mode=DR)

                for it in range(iters):
                    buf = it % 2
                    nth = (it // 2 + 1) * 16  # per-parity DMA count

                    # ---- wave A: v[:, 0:512], banks 0-4 (bank = lt) ----
                    # kp-outer; passes HH (x_hi*w_hi) and LH (x_lo*w_hi)
                    # interleaved per kp to match DMA arrival order; HL
                    # (x_hi*w_lo) trails.
                    for t in range(KP):
                        if t == 0:
                            if it == 0:
                                tensor.wait_ge(sems["xz"], 1)
                            tensor.wait_ge(sems[f"xh01b{buf}"], nth)
                            tensor.wait_ge(sems[f"wah01b{buf}"], nth)
                        elif t == 2:
                            tensor.wait_ge(sems[f"xh23b{buf}"], nth)
                            tensor.wait_ge(sems[f"wah23b{buf}"], nth)
                        for lt in range(LT):
                            if t == 0 and it > 0:
                                # bank lt <- ph3 m=lt copy of prev iter
                                tensor.wait_ge(cp3_sem(lt), cp3_count(lt, it - 1))
                            dr_mm(ps[lt],
                                  xt_sb[:, buf, 0, 2 * t:2 * t + 2,
                                        128 * lt:128 * lt + 128],
                                  w1_sb[:, buf, 0, 2 * t:2 * t + 2, 0:512],
                                  start=(t == 0), stop=False)
                    for t in range(KP):
                        if t == 0:
                            tensor.wait_ge(sems[f"xl01b{buf}"], nth)
                        elif t == 2:
                            tensor.wait_ge(sems[f"xl23b{buf}"], nth)
                        for lt in range(LT):
                            dr_mm(ps[lt],
                                  xt_sb[:, buf, 1, 2 * t:2 * t + 2,
                                        128 * lt:128 * lt + 128],
                                  w1_sb[:, buf, 0, 2 * t:2 * t + 2, 0:512],
                                  start=False, stop=False)
                    for t in range(KP):
                        if t == 0:
                            tensor.wait_ge(sems[f"wal01b{buf}"], nth)
                        elif t == 2:
                            tensor.wait_ge(sems[f"wal23b{buf}"], nth)
                        for lt in range(LT):
                            mm = dr_mm(ps[lt],
                                       xt_sb[:, buf, 0, 2 * t:2 * t + 2,
                                             128 * lt:128 * lt + 128],
                                       w1_sb[:, buf, 1, 2 * t:2 * t + 2, 0:512],
                                       start=False, stop=(t == KP - 1))
                            if t == KP - 1:
                                mm.then_inc(sems["mmA"])

                    # ---- wave B: v[:, 512:1024], banks [5,6,7,0,1], lt-outer --
                    for lt in range(LT):
                        bank = ps[WB[lt]]
                        if it > 0 and lt == 0:
                            tensor.wait_ge(cp3_sem(5), cp3_count(5, it - 1))
                        elif it > 0 and lt == 1:
                            tensor.wait_ge(cp3_sem(6), cp3_count(6, it - 1))
                        elif it > 0 and lt == 2:
                            tensor.wait_ge(cp3_sem(7), cp3_count(7, it - 1))
                        elif lt == 3:
                            tensor.wait_ge(sems["cpA"], it * LT + 1)
                        elif lt == 4:
                            tensor.wait_ge(sems["cpA"], it * LT + 2)
                        for pas in range(3):  # HH, LH, HL
                            sx = 1 if pas == 1 else 0
                            sw = 1 if pas == 2 else 0
                            for t in range(KP):
                                if lt == 0 and pas == 0 and t == 0:
                                    tensor.wait_ge(sems[f"wbhb{buf}"], nth)
                                if lt == 0 and pas == 2 and t == 0:
                                    tensor.wait_ge(sems[f"wblb{buf}"], nth)
                                mm = dr_mm(
                                    bank,
                                    xt_sb[:, buf, sx, 2 * t:2 * t + 2,
                                          128 * lt:128 * lt + 128],
                                    w1_sb[:, buf, sw, 2 * t:2 * t + 2, 512:1024],
                                    start=(pas == 0 and t == 0),
                                    stop=(pas == 2 and t == KP - 1))
                                if pas == 2 and t == KP - 1:
                                    mm.then_inc(sems["mm1"])

                    # ---- phase 2: attendedT per head (bf16 banded) ----
                    if it == 0:
                        tensor.wait_ge(sems["tp_d"], 16)
                    for h in range(H):
                        bank = ps[PH2_BANKS[h]]
                        # bank WAR: banks 2,3,4 <- wave A lt=2,3,4 copies;
                        # bank 5 <- wave B lt=0 copy; banks 0,1 <- wave B
                        # lt=3,4 copies; h>=6 <- ph2 head h-6 copy
                        if h == 0:
                            tensor.wait_ge(sems["cpA"], it * LT + 3)
                        elif h == 1:
                            tensor.wait_ge(sems["cpA"], it * LT + 4)
                        elif h == 2:
                            tensor.wait_ge(sems["cpA"], it * LT + 5)
                        elif h == 3:
                            tensor.wait_ge(sems["cpB"], it * LT + 1)
                        elif h == 4:
                            tensor.wait_ge(sems["cpB"], it * LT + 4)
                        elif h == 5:
                            tensor.wait_ge(sems["cpB"], it * LT + 5)
                        else:
                            tensor.wait_ge(cp2_sem(h - 6), cp2_count(h - 6, it))
                        windows = attn_windows(h)
                        for wi, (t, j0, j1) in enumerate(windows):
                            if h // 4 == 0:
                                tensor.wait_ge(sems["cpA"], it * LT + t + 1)
                            else:
                                tensor.wait_ge(sems["cpB"], it * LT + t + 1)
                            c0 = 512 - 128 * t + j0 - TP0
                            c1 = 512 - 128 * t + j1 - TP0
                            mm = tensor.matmul(
                                bank[:, j0:j1],
                                v_sb[:, t, 128 * h:128 * h + 128],
                                tp_sb[:, h, c0:c1],
                                start=(wi == 0), stop=(wi == len(windows) - 1),
                            )
                            if wi == len(windows) - 1:
                                mm.then_inc(sems["mm2"])

                    # ---- phase 3: outT = W2' @ attendedT, banks 0-7 ----
                    # stage 1 (hi-consuming) is fully kp-outer so the at8-hi
                    # for pair t is needed only ~1.7us*t after ph3 starts,
                    # matching the pace the splits come off the copy engines.
                    # Per kp: HH/HL over m-set A (banks from wave B copies),
                    # then over m-set B (banks from ph2 copies, later).
                    # stage 2 (LH, lo-consuming) is m-outer and closes groups.
                    MSET_A = (6, 7, 4, 5)
                    MSET_B = (0, 1, 2, 3)
                    for t in range(KP):
                        tensor.wait_ge(sems["phiV"], it * 4 + t + 1)
                        tensor.wait_ge(sems["phiP"], it * 4 + t + 1)
                        if t == 0:
                            tensor.wait_ge(sems["w2h"], (it + 1) * 16)
                        for si, mset in enumerate((MSET_A, MSET_B)):
                            for pas in range(2):  # HH, HL
                                if t == 0 and si == 0 and pas == 1:
                                    tensor.wait_ge(sems["w2l"], (it + 1) * 16)
                                for m in mset:
                                    if t == 0 and pas == 0:
                                        # bank WAR: last ph2 user of bank m
                                        if m == 6:
                                            tensor.wait_ge(sems["cpB"],
                                                           it * LT + 2)
                                        elif m == 7:
                                            tensor.wait_ge(sems["cpB"],
                                                           it * LT + 3)
                                        else:
                                            hh = {0: 4, 1: 5, 2: 6, 3: 7,
                                                  4: 2, 5: 3}[m]
                                            tensor.wait_ge(cp2_sem(hh),
                                                           cp2_count(hh, it))
                                    dr_mm(ps[m],
                                          w2_sb[:, pas, 2 * t:2 * t + 2,
                                                128 * m:128 * m + 128],
                                          a8_sb[:, 0, 2 * t:2 * t + 2, :],
                                          start=(t == 0 and pas == 0),
                                          stop=False)
                    for m in range(KT):  # LH pass, closes groups
                        for t in range(KP):
                            if m == 0:
                                tensor.wait_ge(sems["ploV"], it * 4 + t + 1)
                                tensor.wait_ge(sems["ploP"], it * 4 + t + 1)
                            mm = dr_mm(ps[m],
                                       w2_sb[:, 0, 2 * t:2 * t + 2,
                                             128 * m:128 * m + 128],
                                       a8_sb[:, 1, 2 * t:2 * t + 2, :],
                                       start=False, stop=(t == KP - 1))
                            if t == KP - 1:
                                mm.then_inc(sems["mm3"])

            @block.vector
            def _(vector: bass.BassEngine):
                for it in range(iters):
                    # wave A copies: v[:, lt, 0:512] from banks 0-4
                    for lt in range(LT):
                        vector.wait_ge(sems["mmA"], it * LT + lt + 1)
                        vector.tensor_copy(
                            out=v_sb[:, lt, 0:512], in_=ps[lt][:, :],
                        ).then_inc(sems["cpA"])
                    # ph2 even heads: bf16 copies (free banks) + fp8 hi
                    # splits, ordered so cp2v(h4)/cp2v(h6) (ph3 m-set B bank
                    # WARs) are not queued behind hi splits
                    def cp2(h):
                        vector.wait_ge(sems["mm2"], it * H + h + 1)
                        vector.tensor_copy(
                            out=ab_sb[:, h, :], in_=ps[PH2_BANKS[h]][:, :],
                        ).then_inc(sems["cp2v"])

                    def hi(h):
                        vector.tensor_copy(
                            out=a8_sb[:, 0, h, :], in_=ab_sb[:, h, :],
                        ).then_inc(sems["phiV"])

                    cp2(0); hi(0); cp2(2); cp2(4); cp2(6); hi(2); hi(4); hi(6)
                    for h in (0, 2, 4, 6):
                        vector.tensor_sub(
                            a8_sb[:, 1, h, :], ab_sb[:, h, :], a8_sb[:, 0, h, :],
                        ).then_inc(sems["ploV"])
                    for m in (0, 2, 4, 6):
                        vector.wait_ge(sems["mm3"], it * KT + m + 1)
                        if it > 0:
                            vector.wait_ge(sems[f"dmo{m}"], it * 16)
                        vector.tensor_copy(
                            out=o_sb[:, m, :], in_=ps[m][:, :],
                        ).then_inc(sems["cp3v"])

            @block.gpsimd
            def _(gpsimd: bass.BassEngine):
                # zero the PE-warmup tile before anything else
                gpsimd.memset(zdum[:], 0).then_inc(sems["zd"])
                # zero the untransferred xt pad rows (cols 608:640) once
                gpsimd.memset(xt_sb[:, :, :, :, LREAL:LPAD], 0).then_inc(
                    sems["xz"])
                for it in range(iters):
                    # ph2 odd heads: fp8 hi then lo splits (SBUF only)
                    for h in (1, 3, 5, 7):
                        gpsimd.wait_ge(sems["cp2s"], it * 4 + h // 2 + 1)
                        gpsimd.tensor_copy(
                            out=a8_sb[:, 0, h, :], in_=ab_sb[:, h, :],
                        ).then_inc(sems["phiP"])
                    for h in (1, 3, 5, 7):
                        gpsimd.tensor_sub(
                            a8_sb[:, 1, h, :], ab_sb[:, h, :], a8_sb[:, 0, h, :],
                        ).then_inc(sems["ploP"])
                    for m in range(6 if it == iters - 1 else 7):
                        gpsimd.wait_ge(cp3_sem(m), cp3_count(m, it))
                        gpsimd.dma_start(
                            out=out[128 * m:128 * m + 128, :],
                            in_=o_sb[:, m, :],
                        ).then_inc(sems[f"dmo{m}"], 16)

                for m in range(KT):
                    if m == 6:
                        gpsimd.wait_ge(sems["dmo6"], (iters - 1) * 16)
                        gpsimd.wait_ge(sems["dmo6f"], 16)
                    else:
                        gpsimd.wait_ge(sems[f"dmo{m}"], iters * 16)

            @block.scalar
            def _(scalar: bass.BassEngine):
                for it in range(iters):
                    # wave B copies: v[:, lt, 512:1024] from banks [5,6,7,0,1]
                    for lt in range(LT):
                        scalar.wait_ge(sems["mm1"], it * LT + lt + 1)
                        scalar.copy(v_sb[:, lt, 512:1024],
                                    ps[WB[lt]][:, :]).then_inc(sems["cpB"])
                    for h in (1, 3, 5, 7):
                        scalar.wait_ge(sems["mm2"], it * H + h + 1)
                        if it > 0:
                            # ab_sb[h] reuse: prev-iter lo split (Pool) done
                            scalar.wait_ge(sems["ploP"], (it - 1) * 4 + h // 2 + 1)
                        scalar.copy(ab_sb[:, h, :],
                                    ps[PH2_BANKS[h]][:, :]).then_inc(sems["cp2s"])
                    for m in (1, 3, 5, 7):
                        scalar.wait_ge(sems["mm3"], it * KT + m + 1)
                        if it > 0:
                            scalar.wait_ge(sems[f"dmo{m}"], it * 16)
                        scalar.copy(o_sb[:, m, :],
                                    ps[m][:, :]).then_inc(sems["cp3s"])
                        if m == 7:
                            # tail: issue m7's store here, skipping the Pool hop
                            scalar.dma_start(
                                out=out[896:1024, :], in_=o_sb[:, 7, :],
                            ).then_inc(sems["dmo7"], 16)

    return nc


# ---------------- host side ----------------

_GRAPH_CACHE: dict = {}


def get_graph(iters: int = 1, banded: bool = True) -> bass.Bass:
    key = (iters, banded)
    if key not in _GRAPH_CACHE:
        _GRAPH_CACHE[key] = build_graph(iters, banded)
    return _GRAPH_CACHE[key]


class Runner:
    """Compile-once executor for one Bass graph across the 8 cores.

    Mirrors bass2jax.run_bass_via_pjrt but keeps the jitted callable so
    repeated invocations don't re-trace/re-compile.
    """

    def __init__(self, nc: bass.Bass, n_cores: int = N_CORES):
        import jax
        from jax.sharding import Mesh, PartitionSpec
        from jax.experimental.shard_map import shard_map
        from concourse import bass2jax, mybir as _mb

        bass2jax.install_neuronx_cc_hook()
        self.n_cores = n_cores

        partition_name = (nc.partition_id_tensor.name
                          if nc.partition_id_tensor else None)
        in_names, out_names, out_avals, zero_shapes = [], [], [], []
        for alloc in nc.m.functions[0].allocations:
            if not isinstance(alloc, _mb.MemoryLocationSet):
                continue
            name = alloc.memorylocations[0].name
            if alloc.kind == "ExternalInput":
                if name != partition_name:
                    in_names.append(name)
            elif alloc.kind == "ExternalOutput":
                out_names.append(name)
                shape = tuple(alloc.tensor_shape)
                dtype = _mb.dt.np(alloc.dtype)
                out_avals.append(jax.core.ShapedArray(shape, dtype))
                zero_shapes.append((shape, dtype))
        self.in_names = list(in_names)
        self.out_names = out_names
        self.out_avals = out_avals
        self.zero_shapes = zero_shapes
        n_params = len(in_names)
        all_names = in_names + out_names
        if partition_name is not None:
            all_names = all_names + [partition_name]

        def _body(*args):
            operands = list(args)
            if partition_name is not None:
                operands.append(bass2jax.partition_id_tensor())
            outs = bass2jax._bass_exec_p.bind(
                *operands,
                out_avals=tuple(out_avals),
                in_names=tuple(all_names),
                out_names=tuple(out_names),
                lowering_input_output_aliases=(),
                sim_require_finite=True,
                sim_require_nnan=True,
                nc=nc,
            )
            return tuple(outs)

        devices = jax.devices()[:n_cores]
        mesh = Mesh(np.asarray(devices), ("core",))
        self._mesh = mesh
        n_outs = len(out_names)
        self._fn = jax.jit(
            shard_map(_body, mesh=mesh,
                      in_specs=(PartitionSpec("core"),) * (n_params + n_outs),
                      out_specs=(PartitionSpec("core"),) * n_outs,
                      check_rep=False),
            donate_argnums=tuple(range(n_params, n_params + n_outs)),
            keep_unused=True,
        )

    def stage(self, in_maps):
        """device_put the concatenated inputs once; returns device arrays."""
        import jax
        concat_in = [
            np.concatenate([np.asarray(m[name]) for m in in_maps], axis=0)
            for name in self.in_names
        ]
        return [jax.device_put(a) for a in concat_in]

    def make_zeros(self):
        if not hasattr(self, "_zeros_fn"):
            import jax
            import jax.numpy as jnp
            from jax.sharding import NamedSharding, PartitionSpec
            shardings = tuple(
                NamedSharding(self._mesh, PartitionSpec("core"))
                for _ in self.zero_shapes)
            shapes = [((self.n_cores * s[0], *s[1:]), d)
                      for s, d in self.zero_shapes]

            def _mk():
                return tuple(jnp.zeros(sh, dt) for sh, dt in shapes)

            self._zeros_fn = jax.jit(_mk, out_shardings=shardings)
        return list(self._zeros_fn())

    def run_staged(self, dev_in, dev_zeros):
        return self._fn(*dev_in, *dev_zeros)

    def __call__(self, in_maps):
        out_arrs = self._fn(*self.stage(in_maps), *self.make_zeros())
        return [
            {name: np.asarray(out_arrs[i]).reshape(
                self.n_cores, *self.out_avals[i].shape)[c]
             for i, name in enumerate(self.out_names)}
            for c in range(self.n_cores)
        ]


_RUNNER_CACHE: dict = {}


def get_runner(iters: int = 1) -> "Runner":
    if iters not in _RUNNER_CACHE:
        _RUNNER_CACHE[iters] = Runner(get_graph(iters))
    return _RUNNER_CACHE[iters]


def _split8(a: np.ndarray) -> np.ndarray:
    """[2, ...] stack of (hi, lo) fp8e4 halves of a float32 array."""
    hi = a.astype(np.float32).astype(NPF8)
    lo = (a.astype(np.float32) - hi.astype(np.float32)).astype(NPF8)
    return np.stack([hi, lo])


def make_in_maps(values: np.ndarray, input_weights: np.ndarray,
                 output_weight: np.ndarray) -> list:
    w1s = _split8(np.ascontiguousarray(input_weights.T) * S_W)
    w2s = _split8(np.ascontiguousarray(output_weight.T) * S_W)
    tpt = gauss_toeplitz_table()
    in_maps = []
    for core in range(N_CORES):
        b, c = divmod(core, 4)
        lo, hi = c * CHUNK - HALO_L, c * CHUNK + CHUNK + HALO_R
        src_lo, src_hi = max(lo, 0), min(hi, L)
        xt_pad = np.zeros((D, LREAL), dtype=np.float32)
        xt_pad[:, src_lo - lo:src_hi - lo] = values[b, src_lo:src_hi, :].T
        in_maps.append({"xt": _split8(xt_pad), "w1": w1s, "w2": w2s, "tp": tpt})
    return in_maps


def assemble(results: list) -> np.ndarray:
    out = np.empty((B, L, D), dtype=np.float32)
    for core in range(N_CORES):
        b, c = divmod(core, 4)
        out[b, c * CHUNK:(c + 1) * CHUNK, :] = \
            results[core]["out"].T.astype(np.float32) * OUT_DESCALE
    return out


def kernel(values: np.ndarray, input_weights: np.ndarray,
           output_weight: np.ndarray) -> np.ndarray:
    in_maps = make_in_maps(values, input_weights, output_weight)
    try:
        return assemble(get_runner(1)(in_maps))
    except Exception:
        # fallback: canonical SPMD path (re-traces per call but always works)
        res = run_bass_kernel_spmd(get_graph(1), in_maps,
                                   core_ids=list(range(N_CORES)))
        return assemble(res.results)
